# revision 15
# baseline (speedup 1.0000x reference)
"""Trainium2 Bass kernel for nn_Block_24343874633736 (moe_routing).

Transformer block: RMSNorm -> MHA(RoPE) -> residual -> RMSNorm ->
MoE (8 routed experts, top-2, + 1 shared expert) -> residual.

Sharding (8 NeuronCores, single SPMD launch):
  - Attention is HEAD-sharded: every core normalizes all 4096 tokens
    and computes K/V/Q + attention for its 2 of 16 heads over both
    batches. One 8-core AllToAll then swaps (head-dims x token-chunks)
    so each core ends with all 16 heads' attention output for its 512
    local tokens. No K/V collectives and no pre-attention stall; the
    initial collective barrier is fully hidden under compute.
  - K/Q are computed directly in transposed layout (weights stationary,
    tokens on the free axis) with RoPE applied in transposed space via
    a per-head (re-pairs, im-pairs) weight-column permutation; the
    rope elementwise work is split across Vector (q) and GpSimd (k).
  - xnT is produced straight from bf16 x tiles by transposing against
    diag(1/rms) instead of the identity (folds the norm scale into the
    PE transpose).
  - MoE: expert-parallel, one routed expert per core. A tiny fp32
    logits AllGather runs first so the routing/destination computation
    overlaps the large bf16 hn AllGather; the shared expert's matmuls
    fill the gather window. Token compaction per expert uses per-tile
    triangular-matmul cumsums + indirect-DMA scatters into NSPLIT
    DRAM buffers (breaks WAW chains); the host scatter-adds.

Numerics: bf16 TensorE matmuls with fp32 PSUM accumulation everywhere
except the gate logits, which are computed in fp32 so the top-2
selection can be replicated exactly host-side from lg_out.
"""

import sys

for _p in ("/opt/trn_rl_repo",):
    if _p not in sys.path:
        sys.path.insert(0, _p)

import numpy as np
import ml_dtypes

import concourse.bass as bass
import concourse.mybir as mybir
from concourse import bacc
from concourse.masks import make_identity, make_upper_triangular
from concourse.tile import TileContext

BF16 = ml_dtypes.bfloat16
F32 = mybir.dt.float32
BF = mybir.dt.bfloat16
I32 = mybir.dt.int32
AX = mybir.AxisListType
OP = mybir.AluOpType
ACTF = mybir.ActivationFunctionType

P = 128
DIM = 1024
NH = 16
HD = 64
E = 8
HID = 1024
EPS = 1e-6
BIG = 60000.0  # trash slot index (> any capacity; exact in fp32/int32)

B_FULL, S_FULL = 2, 2048
NTOK = B_FULL * S_FULL    # 4096 tokens total
LQ_FULL = 512             # tokens owned per core
C_FULL = 1280             # per-expert token capacity
NHL = 2                   # heads per core


def _ts(i, n):
    return slice(i * n, (i + 1) * n)


def build_nc(LQ=LQ_FULL, C=C_FULL, n_cores=8):
    """Emit the SPMD Bass program. All 8 cores run this same program."""
    NTT = NTOK // P          # 32 token tiles
    NSPAN = NTOK // 512      # 8 projection spans
    NQ = LQ // P             # 4 local tiles
    NCAP = C // P
    ND = DIM // P
    NA = NTOK // P           # 32
    assert NA <= P

    nc = bacc.Bacc("TRN2", target_bir_lowering=False, debug=False,
                   num_devices=n_cores)

    # ---- I/O ----
    xbf_in = nc.dram_tensor("x_bf", [NTOK, DIM], BF, kind="ExternalInput")
    x_in = nc.dram_tensor("x_chunk", [LQ, DIM], F32, kind="ExternalInput")
    cosT_in = nc.dram_tensor("cosT", [P, S_FULL], F32, kind="ExternalInput")
    sinT_in = nc.dram_tensor("sinT", [P, S_FULL], F32, kind="ExternalInput")
    wq_in = nc.dram_tensor("wq_bf", [DIM, P], BF, kind="ExternalInput")
    wk_in = nc.dram_tensor("wk_bf", [DIM, P], BF, kind="ExternalInput")
    wv_in = nc.dram_tensor("wv_bf", [DIM, P], BF, kind="ExternalInput")
    wo_in = nc.dram_tensor("wo_bf", [DIM, DIM], BF, kind="ExternalInput")
    gate_in = nc.dram_tensor("gate32", [DIM, E], F32, kind="ExternalInput")
    sw1_in = nc.dram_tensor("sw1_bf", [DIM, HID], BF, kind="ExternalInput")
    sw2_in = nc.dram_tensor("sw2_bf", [HID, DIM], BF, kind="ExternalInput")
    sw3_in = nc.dram_tensor("sw3_bf", [DIM, HID], BF, kind="ExternalInput")
    ew1_in = nc.dram_tensor("ew1_bf", [DIM, HID], BF, kind="ExternalInput")
    ew2_in = nc.dram_tensor("ew2_bf", [HID, DIM], BF, kind="ExternalInput")
    ew3_in = nc.dram_tensor("ew3_bf", [DIM, HID], BF, kind="ExternalInput")
    oh_in = nc.dram_tensor("onehot", [1, NA * E], F32, kind="ExternalInput")

    out_local = nc.dram_tensor("out_local", [LQ, DIM], F32, kind="ExternalOutput")
    eo_out = nc.dram_tensor("eo_out", [C, DIM], F32, kind="ExternalOutput")
    lg_out = nc.dram_tensor("lg_out", [NTOK, E], F32, kind="ExternalOutput")

    # internal DRAM
    warm_in = nc.dram_tensor("warm_in", [8, 64], BF)
    warm_out = nc.dram_tensor("warm_out", [8, 64], BF)
    att_loc = nc.dram_tensor("att_loc", [n_cores * P, LQ], BF)
    att_recv = nc.dram_tensor("att_recv", [n_cores * P, LQ], BF)
    lg_loc = nc.dram_tensor("lg_loc", [LQ, E], F32)
    lg_full = nc.dram_tensor("lg_full", [NTOK, E], F32, addr_space="Shared")
    hn_loc = nc.dram_tensor("hn_loc", [LQ, DIM], BF)
    hn_full = nc.dram_tensor("hn_full", [NTOK, DIM], BF, addr_space="Shared")
    NSPLIT = 4  # scatter-chain split: breaks the WAW serialization
    ebufs = [nc.dram_tensor(f"ebuf{i}", [C, DIM], BF) for i in range(NSPLIT)]

    all_groups = [list(range(n_cores))]

    from contextlib import ExitStack
    with TileContext(nc) as tc, ExitStack() as stack:
        const_pool = stack.enter_context(tc.tile_pool(name="const", bufs=1))
        id_bf = const_pool.tile([P, P], BF)
        make_identity(nc, id_bf[:])
        id_f32 = const_pool.tile([P, P], F32)
        make_identity(nc, id_f32[:])
        ltri = const_pool.tile([P, P], F32)
        make_upper_triangular(nc, ltri[:], val=1.0, diag=True)  # L[k,p]=1 iff k<=p
        ltri_s = const_pool.tile([P, P], F32)
        make_upper_triangular(nc, ltri_s[:], val=1.0, diag=False)  # k<p
        ones_col = const_pool.tile([P, 1], F32)
        nc.vector.memset(ones_col[:], 1.0)
        eps_col = const_pool.tile([P, 1], F32)
        nc.vector.memset(eps_col[:], EPS)
        oh_bc = const_pool.tile([P, NA * E], F32)
        oh_row = const_pool.tile([1, NA * E], F32)
        nc.sync.dma_start(out=oh_row[:], in_=oh_in[:, :])
        nc.gpsimd.partition_broadcast(oh_bc[:], oh_row[:])

        # warmup AllToAll: pays the cold collective-stream cost under
        # compute so the real att AllToAll later runs warm
        wtile = const_pool.tile([8, 64], BF)
        nc.vector.memset(wtile[:], 0.0)
        nc.sync.dma_start(out=warm_in[:, :], in_=wtile[:])
        nc.gpsimd.collective_compute(
            "AllToAll", OP.bypass, replica_groups=all_groups,
            ins=[warm_in.ap().opt()], outs=[warm_out.ap().opt()])

        # persistent activations
        persist = stack.enter_context(tc.tile_pool(name="persist", bufs=1))
        h_sb = [persist.tile([P, DIM], F32, name=f"h{i}", tag=f"h{i}")
                for i in range(NQ)]
        hnT = [persist.tile([P, LQ], BF, name=f"hnT{j}", tag=f"hnT{j}")
               for j in range(ND)]
        rr_sb = [persist.tile([P, 1], F32, name=f"rr{i}", tag=f"rr{i}")
                 for i in range(NQ)]

        # LIFO-nested scoped pools
        sc_att = ExitStack()   # attT_d (post-A2A), closes after D
        p_att = sc_att.enter_context(tc.tile_pool(name="p_att", bufs=1))
        sc_kv = ExitStack()    # kT/qT/vaug/attT_sb, closes after C/A2A out
        p_kv = sc_kv.enter_context(tc.tile_pool(name="p_kv", bufs=1))
        sc_xnT = ExitStack()   # xnT + rope tables + w slices, closes after B
        p_xnT = sc_xnT.enter_context(tc.tile_pool(name="p_xnT", bufs=1))

        # =============== stages A/B/C: norm, K/V/Q, attention ========
        # Processed per batch so only half the transposed activations are
        # resident; C(batch 0) overlaps A/B(batch 1) on the engine queues.
        scAB = nc.enter_named_scope("ABC_attn", False)
        NSB = S_FULL // 512      # 4 spans per batch
        NTB = S_FULL // P        # 16 token tiles per batch
        xnT = [p_xnT.tile([P, S_FULL], BF, name=f"xnT{j}", tag=f"xnT{j}")
               for j in range(ND)]
        cosT = p_xnT.tile([P, S_FULL], F32, name="cosT", tag="cosT")
        sinT = p_xnT.tile([P, S_FULL], F32, name="sinT", tag="sinT")
        nc.sync.dma_start(out=cosT[:], in_=cosT_in[:, :])
        nc.sync.dma_start(out=sinT[:], in_=sinT_in[:, :])
        # per-head-sliced weights: [1024, 128] -> sbuf [128, 8*128]
        wk_sb = p_xnT.tile([P, DIM], BF, name="wk", tag="wk")
        wq_sb = p_xnT.tile([P, DIM], BF, name="wq", tag="wq")
        wv_sb = p_xnT.tile([P, DIM], BF, name="wv", tag="wv")
        for w_sb, w_in in ((wk_sb, wk_in), (wq_sb, wq_in), (wv_sb, wv_in)):
            nc.scalar.dma_start(
                out=w_sb[:].rearrange("p (j d) -> p j d", j=ND),
                in_=w_in.ap().rearrange("(j p) d -> p j d", p=P))

        kT = p_kv.tile([P, S_FULL], BF, name="kT", tag="kT")
        qT = p_kv.tile([P, S_FULL], BF, name="qT", tag="qT")
        VW = NHL * (HD + 1)  # 130 cols per token tile: (64 v + 1 one) x 2 heads
        vaug = p_kv.tile([P, NTB * VW], BF, name="vaug", tag="vaug")
        attT_sb = p_kv.tile([P, NTOK], BF, name="attT", tag="attT")

        # zero the ebuf split buffers early (cheap queue issues; the DMAs
        # drain long before stage F's scatters)
        zt = const_pool.tile([P, DIM], BF)
        nc.vector.memset(zt[:], 0.0)
        for i in range(NSPLIT):
            for sc_ in range(NCAP):
                nc.scalar.dma_start(out=ebufs[i][_ts(sc_, P), :], in_=zt[:])

        def ropeT(eng, ps, pool, span, outT):  # ps must be SBUF for gpsimd
            """RoPE in transposed (re,im)-permuted space; write bf16 outT.

            sinT carries [+sin, -sin] per 32-row half-block, so the
            swapped product lands pre-signed and every tensor_tensor has
            partition-aligned inputs (BIR requirement):
              out = ps*cos + swap32(ps)*sin_signed
            """
            sl = _ts(span, 512)
            tc_ = pool.tile([P, 512], F32, tag="rp_c")
            ts_ = pool.tile([P, 512], F32, tag="rp_s")
            eng.tensor_tensor(out=tc_[:], in0=ps, in1=cosT[:, sl], op=OP.mult)
            for h in range(NHL):
                re = slice(h * 64, h * 64 + 32)
                im = slice(h * 64 + 32, h * 64 + 64)
                eng.tensor_tensor(out=ts_[re, :], in0=ps[im, :],
                                  in1=sinT[im, sl], op=OP.mult)
                eng.tensor_tensor(out=ts_[im, :], in0=ps[re, :],
                                  in1=sinT[re, sl], op=OP.mult)
            eng.tensor_tensor(out=outT[:, sl], in0=tc_[:], in1=ts_[:],
                              op=OP.add)

        with tc.tile_pool(name="stA", bufs=2) as pa, \
             tc.tile_pool(name="stB", bufs=2) as pb, \
             tc.tile_pool(name="stC", bufs=3) as pc, \
             tc.tile_pool(name="st_ps", bufs=2, space="PSUM") as ps_pool, \
             tc.tile_pool(name="st_pst", bufs=2, space="PSUM") as pst_pool, \
             tc.tile_pool(name="st_av", bufs=1, space="PSUM") as av_pool:

            def projT(w_sb, span):
                ps = ps_pool.tile([P, 1024], F32, space="PSUM", tag="ps")
                for j in range(ND):
                    nc.tensor.matmul(out=ps[:, 0:512], lhsT=w_sb[:, _ts(j, P)],
                                     rhs=xnT[j][:, _ts(span, 512)],
                                     start=(j == 0), stop=(j == ND - 1))
                return ps

            for b in range(B_FULL):
                for s in range(NSB):
                    ssl = _ts(s, 512)
                    # --- A: norm for the 4 token tiles of span s
                    xts, diags = [], []
                    for u in range(4):
                        tg = b * NTB + 4 * s + u
                        xt = pa.tile([P, DIM], BF, tag=f"xt{u}")
                        nc.sync.dma_start(out=xt[:], in_=xbf_in[_ts(tg, P), :])
                        sq = pa.tile([P, DIM], BF, tag="sq")
                        ssq = pa.tile([P, 1], F32, tag="ssq")
                        nc.scalar.activation(out=sq[:], in_=xt[:],
                                             func=ACTF.Square, accum_out=ssq[:])
                        rms = pa.tile([P, 1], F32, tag="rms")
                        nc.scalar.activation(out=rms[:], in_=ssq[:],
                                             func=ACTF.Sqrt,
                                             scale=1.0 / DIM, bias=eps_col[:])
                        rr = pa.tile([P, 1], F32, tag="rr")
                        nc.vector.reciprocal(out=rr[:], in_=rms[:])
                        diag = pa.tile([P, P], BF, tag=f"diag{u}")
                        nc.vector.tensor_scalar_mul(diag[:], id_bf[:], rr[:])
                        xts.append(xt)
                        diags.append(diag)
                    # transposes grouped per j: 4 tiles -> one [128,512] copy
                    for j in range(ND):
                        pst = pst_pool.tile([P, 512], BF, space="PSUM",
                                            tag="pstA")
                        for u in range(4):
                            nc.tensor.transpose(out=pst[:, _ts(u, P)],
                                                in_=xts[u][:, _ts(j, P)],
                                                identity=diags[u][:])
                        nc.vector.tensor_copy(out=xnT[j][:, ssl], in_=pst[:])
                    # --- B: K/Q (rope'd, transposed) + V for span s
                    ps = projT(wk_sb, s)
                    kf = pb.tile([P, 512], F32, tag="kf")
                    nc.scalar.activation(out=kf[:], in_=ps[:, 0:512],
                                         func=ACTF.Copy)
                    ropeT(nc.gpsimd, kf[:], pb, s, kT)
                    ps = projT(wq_sb, s)
                    ropeT(nc.vector, ps[:, 0:512], pb, s, qT)
                    ps = projT(wv_sb, s)
                    vT = pb.tile([P, 512], BF, tag="vT")
                    nc.scalar.activation(out=vT[:], in_=ps[:, 0:512],
                                         func=ACTF.Copy)
                    pst = pst_pool.tile([P, 512], BF, space="PSUM", tag="pstA")
                    for u in range(4):
                        nc.tensor.transpose(out=pst[:, _ts(u, P)],
                                            in_=vT[:, _ts(u, P)],
                                            identity=id_bf[:])
                    # strided copy: psum (u h d) -> vaug (u [h d |1])
                    va4 = vaug[:, s * 4 * VW:(s + 1) * 4 * VW].rearrange(
                        "p (u h d) -> p u h d", u=4, h=NHL)
                    pst4 = pst[:].rearrange("p (u h d) -> p u h d", u=4, h=NHL)
                    nc.vector.tensor_copy(out=va4[:, :, :, 0:HD], in_=pst4)
                    nc.vector.memset(va4[:, :, :, HD:HD + 1], 1.0)

                # --- C: attention for batch b (2 heads, 2 query halves)
                for h in range(NHL):
                    hsl = slice(h * HD, (h + 1) * HD)
                    for qh in range(2):
                        qsl = _ts(qh, 1024)
                        aug = av_pool.tile([HD + 1, 1024], F32, space="PSUM",
                                           tag="aug")

                        def scores(kt):
                            sps = ps_pool.tile([P, 1024], F32, space="PSUM",
                                               tag="ps")
                            for u in range(2):
                                nc.tensor.matmul(
                                    out=sps[:, _ts(u, 512)],
                                    lhsT=kT[hsl, _ts(kt, P)],
                                    rhs=qT[hsl, qh * 1024 + u * 512:
                                           qh * 1024 + (u + 1) * 512],
                                    start=True, stop=True)
                            ex = pc.tile([P, 1024], BF, tag="expT")
                            nc.scalar.activation(out=ex[:], in_=sps[:],
                                                 func=ACTF.Exp)
                            return ex

                        def av(kt, ex):
                            t0 = kt * VW + h * (HD + 1)
                            for u in range(2):
                                nc.tensor.matmul(
                                    out=aug[:, _ts(u, 512)],
                                    lhsT=vaug[:, t0:t0 + HD + 1],
                                    rhs=ex[:, _ts(u, 512)],
                                    start=(kt == 0), stop=(kt == NTB - 1))

                        # software-pipelined: scores(kt+1) issued before
                        # av(kt) so the PE never stalls on the exp
                        prev = scores(0)
                        for kt in range(1, NTB):
                            cur = scores(kt)
                            av(kt - 1, prev)
                            prev = cur
                        av(NTB - 1, prev)
                        rcp = pc.tile([1, 1024], F32, tag="rcp")
                        nc.vector.reciprocal(out=rcp[:], in_=aug[HD:HD + 1, :])
                        rbc = pc.tile([HD, 1024], F32, tag="rbc")
                        nc.gpsimd.partition_broadcast(rbc[:], rcp[:])
                        nc.vector.tensor_tensor(
                            out=attT_sb[hsl, b * S_FULL + qh * 1024:
                                        b * S_FULL + qh * 1024 + 1024],
                            in0=aug[0:HD, :], in1=rbc[:], op=OP.mult)
        sc_xnT.close()
        nc.leave_named_scope("ABC_attn", scAB[0], False)

        # =============== attention AllToAll ==========================
        scA2A = nc.enter_named_scope("A2A_att", False)
        nc.sync.dma_start(
            out=att_loc.ap().rearrange("(c p) t -> p c t", p=P),
            in_=attT_sb[:].rearrange("p (c t) -> p c t", c=n_cores))
        nc.gpsimd.collective_compute(
            "AllToAll", OP.bypass, replica_groups=all_groups,
            ins=[att_loc.ap().opt()], outs=[att_recv.ap().opt()])
        attT = [p_att.tile([P, LQ], BF, name=f"attd{j}", tag=f"attd{j}")
                for j in range(ND)]
        for j in range(ND):
            nc.sync.dma_start(out=attT[j][:], in_=att_recv[_ts(j, P), :])
        sc_kv.close()
        nc.leave_named_scope("A2A_att", scA2A[0], False)

        def load_w(pool, src, tag, slot=None):
            slot = slot or tag
            w = [pool.tile([P, DIM], BF, name=f"{tag}{j}", tag=f"{slot}{j}")
                 for j in range(ND)]
            for j in range(ND):
                nc.sync.dma_start(out=w[j][:], in_=src[_ts(j, P), :])
            return w

        # =============== stage D: O-proj, gate, hn ===================
        scD = nc.enter_named_scope("D_oproj", False)
        with tc.tile_pool(name="stD", bufs=3) as pd, \
             tc.tile_pool(name="stD_w", bufs=1) as pdw, \
             tc.tile_pool(name="stD_ps", bufs=2, space="PSUM") as pd_ps, \
             tc.tile_pool(name="stD_gps", bufs=2, space="PSUM") as pd_gps, \
             tc.tile_pool(name="stD_pst", bufs=2, space="PSUM") as pd_pst, \
             tc.tile_pool(name="stD_hT", bufs=1) as pd_hT:
            wo_sb = load_w(pdw, wo_in, "wo")
            gate_sb = [pdw.tile([P, E], F32, name=f"g32_{j}", tag=f"g32_{j}")
                       for j in range(ND)]
            for j in range(ND):
                nc.sync.dma_start(out=gate_sb[j][:], in_=gate_in[_ts(j, P), :])
            hT32 = [pd_hT.tile([P, LQ], F32, name=f"hT{j}", tag=f"hT{j}")
                    for j in range(ND)]
            for t in range(NQ):
                ps = pd_ps.tile([P, DIM], F32, space="PSUM", tag="ops")
                for half in range(2):
                    for j in range(ND):
                        nc.tensor.matmul(
                            out=ps[:, _ts(half, 512)],
                            lhsT=attT[j][:, _ts(t, P)],
                            rhs=wo_sb[j][:, _ts(half, 512)],
                            start=(j == 0), stop=(j == ND - 1))
                xres = pd.tile([P, DIM], F32, tag="xres")
                nc.sync.dma_start(out=xres[:], in_=x_in[_ts(t, P), :])
                nc.vector.tensor_tensor(out=h_sb[t][:], in0=ps[:],
                                        in1=xres[:], op=OP.add)
                for j in range(ND):
                    pst = pd_pst.tile([P, P], F32, space="PSUM", tag="pstD")
                    nc.tensor.transpose(out=pst[:], in_=h_sb[t][:, _ts(j, P)],
                                        identity=id_f32[:])
                    nc.vector.tensor_copy(out=hT32[j][:, _ts(t, P)], in_=pst[:])
            # fp32 gate logits first (so the tiny logits AllGather can
            # fire early and routing overlaps the hn AllGather)
            for t in range(NQ):
                gps = pd_gps.tile([P, E], F32, space="PSUM", tag="gps")
                for j in range(ND):
                    nc.tensor.matmul(out=gps[:], lhsT=hT32[j][:, _ts(t, P)],
                                     rhs=gate_sb[j][:],
                                     start=(j == 0), stop=(j == ND - 1))
                sq = pd.tile([P, DIM], F32, tag="sqD")
                ssq = pd.tile([P, 1], F32, tag="ssqD")
                nc.scalar.activation(out=sq[:], in_=h_sb[t][:], func=ACTF.Square,
                                     accum_out=ssq[:])
                rms = pd.tile([P, 1], F32, tag="rmsD")
                nc.scalar.activation(out=rms[:], in_=ssq[:], func=ACTF.Sqrt,
                                     scale=1.0 / DIM, bias=eps_col[:])
                nc.vector.reciprocal(out=rr_sb[t][:], in_=rms[:])
                lg = pd.tile([P, E], F32, tag="lg")
                nc.vector.tensor_scalar_mul(lg[:], gps[:], rr_sb[t][:])
                nc.sync.dma_start(out=lg_loc[_ts(t, P), :], in_=lg[:])
            nc.gpsimd.collective_compute(
                "AllGather", OP.bypass, replica_groups=all_groups,
                ins=[lg_loc.ap().opt()], outs=[lg_full.ap().opt()])
            for t in range(NQ):
                hn = pd.tile([P, DIM], BF, tag="hnD")
                nc.scalar.activation(out=hn[:], in_=h_sb[t][:], func=ACTF.Copy,
                                     scale=rr_sb[t][:])
                nc.sync.dma_start(out=hn_loc[_ts(t, P), :], in_=hn[:])
                for j in range(ND):
                    pst = pd_pst.tile([P, P], BF, space="PSUM", tag="pstD")
                    nc.tensor.transpose(out=pst[:], in_=hn[:, _ts(j, P)],
                                        identity=id_bf[:])
                    nc.vector.tensor_copy(out=hnT[j][:, _ts(t, P)], in_=pst[:])
        sc_att.close()
        nc.leave_named_scope("D_oproj", scD[0], False)

        # =============== hn AllGather ================================
        scCC = nc.enter_named_scope("CC_gather", False)
        nc.gpsimd.collective_compute(
            "AllGather", OP.bypass, replica_groups=all_groups,
            ins=[hn_loc.ap().opt()], outs=[hn_full.ap().opt()])
        nc.leave_named_scope("CC_gather", scCC[0], False)

        # =============== stage H: shared expert + local output =======
        # (independent of the gathers: its TensorE work fills the window)
        scH = nc.enter_named_scope("H_shared", False)
        with tc.tile_pool(name="stH", bufs=3) as ph, \
             tc.tile_pool(name="stH_w", bufs=1) as phw, \
             tc.tile_pool(name="stH_gT", bufs=1) as ph_gT, \
             tc.tile_pool(name="stH_ps", bufs=2, space="PSUM") as ph_ps:
            s1_sb = load_w(phw, sw1_in, "s1")
            s3_sb = load_w(phw, sw3_in, "s3")
            gsT = [ph_gT.tile([P, LQ], BF, name=f"gsT{j}", tag=f"gsT{j}")
                   for j in range(ND)]
            for j in range(ND):
                h1 = ph_ps.tile([P, LQ], F32, space="PSUM", tag="sh1")
                h3 = ph_ps.tile([P, LQ], F32, space="PSUM", tag="sh3")
                for d in range(ND):
                    nc.tensor.matmul(out=h1[:], lhsT=s1_sb[d][:, _ts(j, P)],
                                     rhs=hnT[d][:, :],
                                     start=(d == 0), stop=(d == ND - 1))
                for d in range(ND):
                    nc.tensor.matmul(out=h3[:], lhsT=s3_sb[d][:, _ts(j, P)],
                                     rhs=hnT[d][:, :],
                                     start=(d == 0), stop=(d == ND - 1))
                sig = ph.tile([P, LQ], F32, tag="sigH")
                nc.scalar.activation(out=sig[:], in_=h1[:], func=ACTF.Sigmoid)
                nc.vector.tensor_tensor(out=sig[:], in0=sig[:], in1=h1[:],
                                        op=OP.mult)
                nc.vector.tensor_tensor(out=gsT[j][:], in0=sig[:], in1=h3[:],
                                        op=OP.mult)
            s2_sb = load_w(phw, sw2_in, "s2", slot="s1")
            for t in range(NQ):
                ps = ph_ps.tile([P, DIM], F32, space="PSUM", tag="shps")
                for half in range(2):
                    for j in range(ND):
                        nc.tensor.matmul(
                            out=ps[:, _ts(half, 512)],
                            lhsT=gsT[j][:, _ts(t, P)],
                            rhs=s2_sb[j][:, _ts(half, 512)],
                            start=(j == 0), stop=(j == ND - 1))
                ot = ph.tile([P, DIM], F32, tag="ot")
                nc.vector.tensor_tensor(out=ot[:], in0=ps[:], in1=h_sb[t][:],
                                        op=OP.add)
                nc.sync.dma_start(out=out_local[_ts(t, P), :], in_=ot[:])
        nc.leave_named_scope("H_shared", scH[0], False)

        # =============== stage F: routing + dispatch =================
        # Selection on raw fp32 logits (host replicates it from lg_out).
        scF = nc.enter_named_scope("F_route", False)
        with tc.tile_pool(name="stF", bufs=8) as pf, \
             tc.tile_pool(name="stF_keep", bufs=1) as pfk, \
             tc.tile_pool(name="stF_ps", bufs=2, space="PSUM") as pf_ps, \
             tc.tile_pool(name="stF_tot", bufs=1, space="PSUM") as pf_tot:
            lg_all = pfk.tile([P, NA * E], F32)
            nc.sync.dma_start(
                out=lg_all[:].rearrange("p (t e) -> p t e", t=NA),
                in_=lg_full.ap().rearrange("(t p) e -> p t e", p=P))
            nc.sync.dma_start(
                out=lg_out.ap().rearrange("(t p) e -> p t e", p=P),
                in_=lg_all[:].rearrange("p (t e) -> p t e", t=NA))
            v3 = lg_all[:].rearrange("p (t e) -> p t e", t=NA)
            m1 = pfk.tile([P, NA], F32)
            nc.vector.reduce_max(out=m1[:], in_=v3, axis=AX.X)
            ge1 = pfk.tile([P, NA * E], F32)
            g13 = ge1[:].rearrange("p (t e) -> p t e", t=NA)
            nc.vector.tensor_tensor(out=g13, in0=v3,
                                    in1=m1[:, :, None].to_broadcast([P, NA, E]),
                                    op=OP.is_ge)
            msk = pfk.tile([P, NA * E], F32)
            nc.vector.tensor_scalar_mul(msk[:], ge1[:], -1.0e30)
            nc.vector.tensor_tensor(out=msk[:], in0=msk[:], in1=lg_all[:],
                                    op=OP.add)
            m2 = pfk.tile([P, NA], F32)
            nc.vector.reduce_max(out=m2[:],
                                 in_=msk[:].rearrange("p (t e) -> p t e", t=NA),
                                 axis=AX.X)
            ge = pfk.tile([P, NA * E], F32)
            ge3 = ge[:].rearrange("p (t e) -> p t e", t=NA)
            nc.vector.tensor_tensor(out=ge3, in0=v3,
                                    in1=m2[:, :, None].to_broadcast([P, NA, E]),
                                    op=OP.is_ge)
            msel = pfk.tile([P, NA * E], F32)
            nc.vector.tensor_tensor(out=msel[:], in0=ge[:], in1=oh_bc[:],
                                    op=OP.mult)
            ind = pfk.tile([P, NA], F32)
            nc.vector.reduce_sum(out=ind[:],
                                 in_=msel[:].rearrange("p (t e) -> p t e", t=NA),
                                 axis=AX.X)
            # per-tile totals + within-tile inclusive cumsum: one matmul each
            tots = pf_tot.tile([1, NA], F32, space="PSUM")
            nc.tensor.matmul(out=tots[:], lhsT=ones_col[:], rhs=ind[:],
                             start=True, stop=True)
            cnts = pf_tot.tile([P, NA], F32, space="PSUM")
            nc.tensor.matmul(out=cnts[:], lhsT=ltri[:], rhs=ind[:],
                             start=True, stop=True)
            # batched exclusive cumsum of tile totals -> per-tile bases
            tots_sb = pf.tile([1, NA], F32, tag="tots_sb")
            nc.vector.tensor_copy(out=tots_sb[:], in_=tots[:])
            totsT_ps = pf_ps.tile([NA, 1], F32, space="PSUM", tag="totsT", bufs=1)
            nc.tensor.transpose(out=totsT_ps[:], in_=tots_sb[:],
                                identity=id_f32[:1, :1])
            totsT = pf.tile([NA, 1], F32, tag="totsT_sb")
            nc.vector.tensor_copy(out=totsT[:], in_=totsT_ps[:])
            basesT_ps = pf_ps.tile([NA, 1], F32, space="PSUM", tag="basesT", bufs=1)
            nc.tensor.matmul(out=basesT_ps[:], lhsT=ltri_s[:NA, :NA],
                             rhs=totsT[:], start=True, stop=True)
            basesT = pf.tile([NA, 1], F32, tag="basesT_sb")
            nc.vector.tensor_copy(out=basesT[:], in_=basesT_ps[:])
            bases_ps = pf_ps.tile([1, NA], F32, space="PSUM", tag="bases", bufs=1)
            nc.tensor.transpose(out=bases_ps[:], in_=basesT[:],
                                identity=id_f32[:NA, :NA])
            bases_sb = pf.tile([1, NA], F32, tag="bases_sb")
            nc.vector.tensor_copy(out=bases_sb[:], in_=bases_ps[:])
            bb_all = pfk.tile([P, NA], F32)
            nc.gpsimd.partition_broadcast(bb_all[:], bases_sb[:])
            # destinations (batched)
            d_all = pfk.tile([P, NA], F32)
            nc.vector.scalar_tensor_tensor(
                out=d_all[:], in0=cnts[:], scalar=-(1.0 + BIG),
                in1=bb_all[:], op0=OP.add, op1=OP.add)
            nc.vector.tensor_tensor(out=d_all[:], in0=d_all[:], in1=ind[:],
                                    op=OP.mult)
            nc.vector.tensor_scalar_add(d_all[:], d_all[:], BIG)
            dest_all = pfk.tile([P, NA], I32)
            nc.vector.tensor_copy(out=dest_all[:], in_=d_all[:])
            # scatters (independent per tile)
            for t in range(NA):
                hnt = pf.tile([P, DIM], BF, tag="hnF")
                nc.sync.dma_start(out=hnt[:], in_=hn_full[_ts(t, P), :])
                nc.gpsimd.indirect_dma_start(
                    out=ebufs[t % NSPLIT][:, :],
                    out_offset=bass.IndirectOffsetOnAxis(
                        ap=dest_all[:, t:t + 1], axis=0),
                    in_=hnt[:], in_offset=None,
                    bounds_check=C - 1, oob_is_err=False)
        nc.leave_named_scope("F_route", scF[0], False)

        # =============== stage G: expert FFN =========================
        scG = nc.enter_named_scope("G_expert", False)
        with tc.tile_pool(name="stG", bufs=3) as pg, \
             tc.tile_pool(name="stG_w", bufs=1) as pgw, \
             tc.tile_pool(name="stG_gT", bufs=1) as pg_gT:
            ebT = [pg_gT.tile([P, C], BF, name=f"ebT{j}", tag=f"ebT{j}")
                   for j in range(ND)]
            with tc.tile_pool(name="stG_ps", bufs=4, space="PSUM") as pg_ps:
                for s in range(NCAP):
                    parts = []
                    for i in range(NSPLIT):
                        pt = pg.tile([P, DIM], BF, tag=f"ebp{i}", bufs=2)
                        nc.sync.dma_start(out=pt[:], in_=ebufs[i][_ts(s, P), :])
                        parts.append(pt)
                    nc.vector.tensor_tensor(out=parts[0][:], in0=parts[0][:],
                                            in1=parts[1][:], op=OP.add)
                    nc.vector.tensor_tensor(out=parts[2][:], in0=parts[2][:],
                                            in1=parts[3][:], op=OP.add)
                    eb = pg.tile([P, DIM], BF, tag="eb")
                    nc.vector.tensor_tensor(out=eb[:], in0=parts[0][:],
                                            in1=parts[2][:], op=OP.add)
                    for j in range(ND):
                        pst = pg_ps.tile([P, P], BF, space="PSUM", tag="pstG")
                        nc.tensor.transpose(out=pst[:], in_=eb[:, _ts(j, P)],
                                            identity=id_bf[:])
                        nc.vector.tensor_copy(out=ebT[j][:, _ts(s, P)], in_=pst[:])
            e1_sb = load_w(pgw, ew1_in, "e1")
            e3_sb = load_w(pgw, ew3_in, "e3")
            gT = [pg_gT.tile([P, C], BF, name=f"gT{j}", tag=f"gT{j}")
                  for j in range(ND)]
            nsub = (C + 511) // 512
            with tc.tile_pool(name="stG_ps2", bufs=2, space="PSUM") as pg_ps2:
                for j in range(ND):
                    for s in range(nsub):
                        w = min(512, C - s * 512)
                        sl = slice(s * 512, s * 512 + w)
                        h1 = pg_ps2.tile([P, 512], F32, space="PSUM", tag="h1")
                        h3 = pg_ps2.tile([P, 512], F32, space="PSUM", tag="h3")
                        for d in range(ND):
                            nc.tensor.matmul(out=h1[:, :w],
                                             lhsT=e1_sb[d][:, _ts(j, P)],
                                             rhs=ebT[d][:, sl],
                                             start=(d == 0), stop=(d == ND - 1))
                        for d in range(ND):
                            nc.tensor.matmul(out=h3[:, :w],
                                             lhsT=e3_sb[d][:, _ts(j, P)],
                                             rhs=ebT[d][:, sl],
                                             start=(d == 0), stop=(d == ND - 1))
                        sig = pg.tile([P, 512], F32, tag="sig")
                        nc.scalar.activation(out=sig[:, :w], in_=h1[:, :w],
                                             func=ACTF.Sigmoid)
                        nc.vector.tensor_tensor(out=sig[:, :w], in0=sig[:, :w],
                                                in1=h1[:, :w], op=OP.mult)
                        nc.vector.tensor_tensor(out=gT[j][:, sl], in0=sig[:, :w],
                                                in1=h3[:, :w], op=OP.mult)
                e2_sb = load_w(pgw, ew2_in, "e2", slot="e1")
                for s in range(NCAP):
                    ps = pg_ps2.tile([P, DIM], F32, space="PSUM", tag="eops")
                    for half in range(2):
                        for j in range(ND):
                            nc.tensor.matmul(
                                out=ps[:, _ts(half, 512)],
                                lhsT=gT[j][:, _ts(s, P)],
                                rhs=e2_sb[j][:, _ts(half, 512)],
                                start=(j == 0), stop=(j == ND - 1))
                    eo = pg.tile([P, DIM], F32, tag="eo")
                    nc.vector.tensor_copy(out=eo[:], in_=ps[:])
                    nc.sync.dma_start(out=eo_out[_ts(s, P), :], in_=eo[:])
        nc.leave_named_scope("G_expert", scG[0], False)

    nc.compile()
    return nc


# ----------------------------------------------------------------------
# host side
# ----------------------------------------------------------------------

def prep_inputs(x, freqs, att_norm_w, wq, wk, wv, wo, ffn_norm_w, gate_w,
                ew1, ew2, ew3, sw1, sw2, sw3, LQ=LQ_FULL, n_cores=8):
    """Build the 8 per-core input maps (host-side weight folding + slicing)."""
    def tobf(a):
        return np.ascontiguousarray(np.asarray(a, np.float32).astype(BF16))

    B, S, _ = x.shape
    N = B * S
    anw = np.asarray(att_norm_w, np.float32)
    fnw = np.asarray(ffn_norm_w, np.float32)
    wq_n = (anw[:, None] * np.asarray(wq, np.float32)) / np.sqrt(HD)
    wk_n = anw[:, None] * np.asarray(wk, np.float32)
    wv_n = anw[:, None] * np.asarray(wv, np.float32)
    wo_e = tobf(wo)
    gate32 = np.ascontiguousarray((np.asarray(gate_w, np.float32) * fnw[None, :]).T)
    ew1_e = tobf(np.asarray(ew1) * fnw[None, :, None])
    ew3_e = tobf(np.asarray(ew3) * fnw[None, :, None])
    ew2_e = tobf(ew2)
    sw1_e = tobf(np.asarray(sw1) * fnw[:, None])
    sw3_e = tobf(np.asarray(sw3) * fnw[:, None])
    sw2_e = tobf(sw2)

    x_flat = np.asarray(x, np.float32).reshape(N, DIM)
    x_bf = np.ascontiguousarray(x_flat.astype(BF16))
    # rope tables in transposed space: row r -> pair (r % 32)
    cos32 = np.asarray(freqs[:S, :, 0], np.float32).T        # (32, S)
    sin32 = np.asarray(freqs[:S, :, 1], np.float32).T
    cosT = np.ascontiguousarray(np.tile(cos32, (4, 1)))      # (128, S)
    # signed sin: +sin on re rows (0-31 of each head block), -sin on im rows
    sinT = np.ascontiguousarray(np.tile(np.vstack([sin32, -sin32]), (2, 1)))
    # within-head (re, im) column permutation for transposed-space rope
    pidx = np.concatenate([np.arange(0, HD, 2), np.arange(1, HD, 2)])

    in_maps = []
    for core in range(n_cores):
        heads = [2 * core, 2 * core + 1]
        wq_c = np.hstack([wq_n[:, h * HD + pidx] for h in heads])
        wk_c = np.hstack([wk_n[:, h * HD + pidx] for h in heads])
        wv_c = np.hstack([wv_n[:, _ts(h, HD)] for h in heads])
        na = N // 128
        oh = np.zeros((1, E), np.float32)
        oh[0, core % E] = 1.0
        oh = np.tile(oh, (1, na))
        in_maps.append(dict(
            x_bf=x_bf,
            x_chunk=np.ascontiguousarray(x_flat[core * LQ:(core + 1) * LQ]),
            cosT=cosT, sinT=sinT,
            wq_bf=tobf(wq_c), wk_bf=tobf(wk_c), wv_bf=tobf(wv_c),
            wo_bf=wo_e, gate32=gate32,
            sw1_bf=sw1_e, sw2_bf=sw2_e, sw3_bf=sw3_e,
            ew1_bf=ew1_e[core % E], ew2_bf=ew2_e[core % E],
            ew3_bf=ew3_e[core % E],
            onehot=oh,
        ))
    return in_maps


def assemble(results, B, S, LQ=LQ_FULL, n_cores=8):
    N = B * S
    out = np.zeros((N, DIM), np.float32)
    y = np.zeros((N, DIM), np.float32)
    # replicate the device's top-2 selection exactly from the fp32 logits
    lg = np.asarray(results[0]["lg_out"], np.float32)          # (N, E)
    m2 = np.partition(lg, -2, axis=1)[:, -2]
    sel_mask = lg >= m2[:, None]
    ex = np.exp(lg - lg.max(axis=1, keepdims=True), dtype=np.float32)
    probs = ex / ex.sum(axis=1, keepdims=True, dtype=np.float32)
    for core, res in enumerate(results):
        tok0 = core * LQ
        out[tok0:tok0 + LQ] = res["out_local"]
        e = core % E
        sel = np.nonzero(sel_mask[:, e])[0]
        cnt = len(sel)
        eo = res["eo_out"]
        assert cnt <= eo.shape[0], (core, cnt)
        y[sel] += probs[sel, e:e + 1] * eo[:cnt]
    return (out + y).reshape(B, S, DIM)


_NC_CACHE = {}


def kernel(**inputs):
    key = "full"
    if key not in _NC_CACHE:
        _NC_CACHE[key] = build_nc()
    nc = _NC_CACHE[key]
    from concourse.bass_utils import run_bass_kernel_spmd
    in_maps = prep_inputs(**inputs)
    res = run_bass_kernel_spmd(nc, in_maps, core_ids=list(range(8)))
    x = np.asarray(inputs["x"])
    return assemble(res.results, x.shape[0], x.shape[1]).astype(np.float32)


if __name__ == "__main__":
    nc = build_nc()
    print("built + compiled OK")


# revision 36
# speedup vs baseline: 1.0984x; 1.0984x over previous
"""Trainium2 Bass kernel for nn_Block_24343874633736 (moe_routing).

Transformer block: RMSNorm -> MHA(RoPE) -> residual -> RMSNorm ->
MoE (8 routed experts, top-2, + 1 shared expert) -> residual.

Sharding (8 NeuronCores, single SPMD launch):
  - Attention is HEAD-sharded: every core normalizes all 4096 tokens
    and computes K/V/Q + attention for its 2 of 16 heads over both
    batches (processed batch-by-batch to halve SBUF residency). One
    8-core AllToAll then swaps (head-dims x token-chunks) so each core
    ends with all 16 heads' attention output for its 512 local tokens.
    No K/V collectives; the initial collective barrier hides under
    compute.
  - K/Q are computed directly in transposed layout (weights stationary,
    tokens on the free axis) with RoPE applied in transposed space via
    a per-head (re-pairs, im-pairs) weight-column permutation and a
    sign-folded sin table; rope is split across Vector (q) and
    GpSimd (k).
  - MoE: expert-parallel, one routed expert per core. A tiny fp32
    logits AllGather fires first so routing overlaps the bf16 hn
    AllGather; the shared expert fills the gather window. Compaction
    uses triangular-matmul cumsums + indirect-DMA scatters into NSPLIT
    DRAM buffers; the host scatter-adds.

Numerics: everything upstream of the gate logits (norm, Q/K/V,
attention, O-proj) is bf16 with fp32 PSUM so the fp32 gate logits stay
close to the reference and near-tie top-2 flips stay rare (each flip
is a large localized error). The y-path (shared + routed expert FFNs),
which cannot flip routing, runs fp8e4m3 with DoubleRow (0.5 cyc/row);
those weights are pre-scaled x8 host-side to clear the fp8 denormal
range, with descales folded into activation-scale parameters. The host
replicates top-2 selection exactly from lg_out.
"""

import sys

for _p in ("/opt/trn_rl_repo",):
    if _p not in sys.path:
        sys.path.insert(0, _p)

import numpy as np
import ml_dtypes

import concourse.bass as bass
import concourse.mybir as mybir
from concourse import bacc
from concourse.masks import make_identity, make_upper_triangular
from concourse.tile import TileContext

BF16 = ml_dtypes.bfloat16
FP8 = ml_dtypes.float8_e4m3
F32 = mybir.dt.float32
BF = mybir.dt.bfloat16
E4 = mybir.dt.float8e4
I32 = mybir.dt.int32
AX = mybir.AxisListType
OP = mybir.AluOpType
ACTF = mybir.ActivationFunctionType
DR = mybir.MatmulPerfMode.DoubleRow

P = 128
DIM = 1024
NH = 16
HD = 64
E = 8
HID = 1024
EPS = 1e-6
BIG = 60000.0  # trash slot index (> any capacity; exact in fp32/int32)

B_FULL, S_FULL = 2, 2048
NTOK = B_FULL * S_FULL    # 4096 tokens total
LQ_FULL = 512             # tokens owned per core
C_FULL = 1152             # per-expert token capacity (fp64 max count 1062)
NHL = 2                   # heads per core

# fp8 weight pre-scale for the expert FFNs (clears the denormal range)
SW = 8.0


def _ts(i, n):
    return slice(i * n, (i + 1) * n)


def build_nc(LQ=LQ_FULL, C=C_FULL, n_cores=8):
    """Emit the SPMD Bass program. All 8 cores run this same program."""
    NSB = S_FULL // 512      # 4 projection spans per batch
    NTB = S_FULL // P        # 16 token tiles per batch
    NQ = LQ // P             # 4 local tiles
    NCAP = C // P
    ND = DIM // P
    NA = NTOK // P           # 32
    assert NA <= P

    nc = bacc.Bacc("TRN2", target_bir_lowering=False, debug=False,
                   num_devices=n_cores)

    # ---- I/O (weights arrive 128-row pre-tiled: [P, ntiles*cols]) ----
    xbf_in = nc.dram_tensor("x_bf", [NTOK, DIM], BF, kind="ExternalInput")
    x_in = nc.dram_tensor("x_chunk", [LQ, DIM], F32, kind="ExternalInput")
    cosT_in = nc.dram_tensor("cosT", [P, S_FULL], F32, kind="ExternalInput")
    sinT_in = nc.dram_tensor("sinT", [P, S_FULL], F32, kind="ExternalInput")
    wq_in = nc.dram_tensor("wq_t8", [P, DIM], BF, kind="ExternalInput")
    wk_in = nc.dram_tensor("wk_t8", [P, DIM], BF, kind="ExternalInput")
    wv_in = nc.dram_tensor("wv_t8", [P, DIM], BF, kind="ExternalInput")
    wo_in = nc.dram_tensor("wo_t8", [P, ND * DIM], BF, kind="ExternalInput")
    gate_in = nc.dram_tensor("gate_t32", [P, ND * E], F32, kind="ExternalInput")
    sw1_in = nc.dram_tensor("sw1_t8", [P, ND * HID], E4, kind="ExternalInput")
    sw2_in = nc.dram_tensor("sw2_t8", [P, ND * DIM], E4, kind="ExternalInput")
    sw3_in = nc.dram_tensor("sw3_t8", [P, ND * HID], E4, kind="ExternalInput")
    ew1_in = nc.dram_tensor("ew1_t8", [P, ND * HID], E4, kind="ExternalInput")
    ew2_in = nc.dram_tensor("ew2_t8", [P, ND * DIM], E4, kind="ExternalInput")
    ew3_in = nc.dram_tensor("ew3_t8", [P, ND * HID], E4, kind="ExternalInput")
    oh_in = nc.dram_tensor("onehot", [1, NA * E], F32, kind="ExternalInput")

    out_local = nc.dram_tensor("out_local", [LQ, DIM], F32, kind="ExternalOutput")
    eo_out = nc.dram_tensor("eo_out", [C, DIM], F32, kind="ExternalOutput")
    lg_out = nc.dram_tensor("lg_out", [NTOK, E], F32, kind="ExternalOutput")

    # internal DRAM
    warm_in = nc.dram_tensor("warm_in", [8, 64], BF)
    warm_out = nc.dram_tensor("warm_out", [8, 64], BF)
    att_loc = nc.dram_tensor("att_loc", [n_cores * P, LQ], BF)
    att_recv = nc.dram_tensor("att_recv", [n_cores * P, LQ], BF)
    lg_loc = nc.dram_tensor("lg_loc", [LQ, E], F32)
    lg_full = nc.dram_tensor("lg_full", [NTOK, E], F32, addr_space="Shared")
    hn_loc = nc.dram_tensor("hn_loc", [LQ, DIM], BF)
    hn_full = nc.dram_tensor("hn_full", [NTOK, DIM], BF, addr_space="Shared")
    NSPLIT = 4  # scatter-chain split: breaks the WAW serialization
    ebufs = [nc.dram_tensor(f"ebuf{i}", [C, DIM], BF) for i in range(NSPLIT)]

    all_groups = [list(range(n_cores))]

    def r3(tile, n, width):
        return tile[:].rearrange("p (j c) -> p j c", j=n)

    from contextlib import ExitStack
    with TileContext(nc) as tc, ExitStack() as stack:
        const_pool = stack.enter_context(tc.tile_pool(name="const", bufs=1))
        id_bf = const_pool.tile([P, P], BF)
        make_identity(nc, id_bf[:])
        id_f32 = const_pool.tile([P, P], F32)
        make_identity(nc, id_f32[:])
        ltri = const_pool.tile([P, P], F32)
        make_upper_triangular(nc, ltri[:], val=1.0, diag=True)  # L[k,p]=1 iff k<=p
        ltri_s = const_pool.tile([P, P], F32)
        make_upper_triangular(nc, ltri_s[:], val=1.0, diag=False)  # k<p
        ones_col = const_pool.tile([P, 1], F32)
        nc.vector.memset(ones_col[:], 1.0)
        eps_col = const_pool.tile([P, 1], F32)
        nc.vector.memset(eps_col[:], EPS)
        oh_bc = const_pool.tile([P, NA * E], F32)
        oh_row = const_pool.tile([1, NA * E], F32)
        nc.sync.dma_start(out=oh_row[:], in_=oh_in[:, :])
        nc.gpsimd.partition_broadcast(oh_bc[:], oh_row[:])

        # warmup AllToAll: pays the cold collective-stream cost under
        # compute so the real att AllToAll later runs warm
        wtile = const_pool.tile([8, 64], BF)
        nc.vector.memset(wtile[:], 0.0)
        nc.sync.dma_start(out=warm_in[:, :], in_=wtile[:])
        nc.gpsimd.collective_compute(
            "AllToAll", OP.bypass, replica_groups=all_groups,
            ins=[warm_in.ap().opt()], outs=[warm_out.ap().opt()])

        # persistent activations
        persist = stack.enter_context(tc.tile_pool(name="persist", bufs=1))
        h_sb = [persist.tile([P, DIM], F32, name=f"h{i}", tag=f"h{i}")
                for i in range(NQ)]
        hnT = persist.tile([P, ND * LQ], E4, name="hnT", tag="hnT")
        rr_sb = [persist.tile([P, 1], F32, name=f"rr{i}", tag=f"rr{i}")
                 for i in range(NQ)]

        # LIFO-nested scoped pools
        sc_att = ExitStack()   # attd (post-A2A), closes after D
        p_att = sc_att.enter_context(tc.tile_pool(name="p_att", bufs=1))
        sc_kv = ExitStack()    # kT/qT/vaug/attT_sb, closes after A2A send
        p_kv = sc_kv.enter_context(tc.tile_pool(name="p_kv", bufs=1))
        sc_xnT = ExitStack()   # xnT + rope tables + w slices, closes after B
        p_xnT = sc_xnT.enter_context(tc.tile_pool(name="p_xnT", bufs=1))

        # =============== stages A/B/C: norm, K/V/Q, attention ========
        scAB = nc.enter_named_scope("ABC_attn", False)
        xnT = p_xnT.tile([P, ND * S_FULL], BF, name="xnT", tag="xnT")
        xnT3 = xnT[:].rearrange("p (j t) -> p j t", j=ND)
        cosT = p_xnT.tile([P, S_FULL], F32, name="cosT", tag="cosT")
        sinT = p_xnT.tile([P, S_FULL], F32, name="sinT", tag="sinT")
        nc.sync.dma_start(out=cosT[:], in_=cosT_in[:, :])
        nc.sync.dma_start(out=sinT[:], in_=sinT_in[:, :])
        wk_sb = p_xnT.tile([P, DIM], BF, name="wk", tag="wk")
        wq_sb = p_xnT.tile([P, DIM], BF, name="wq", tag="wq")
        wv_sb = p_xnT.tile([P, DIM], BF, name="wv", tag="wv")
        for w_sb, w_in in ((wk_sb, wk_in), (wq_sb, wq_in), (wv_sb, wv_in)):
            nc.scalar.dma_start(out=w_sb[:], in_=w_in[:, :])

        kT = p_kv.tile([P, S_FULL], BF, name="kT", tag="kT")
        qT = p_kv.tile([P, S_FULL], BF, name="qT", tag="qT")
        VW = NHL * (HD + 1)  # 130 cols per token tile: (64 v + 1 one) x 2 heads
        vaug = p_kv.tile([P, NTB * VW], BF, name="vaug", tag="vaug")
        attT_sb = p_kv.tile([P, NTOK], BF, name="attT", tag="attT")

        # zero the ebuf split buffers early (cheap queue issues; the DMAs
        # drain long before stage F's scatters)
        zt = const_pool.tile([P, DIM], BF)
        nc.vector.memset(zt[:], 0.0)
        for i in range(NSPLIT):
            for sc_ in range(NCAP):
                nc.scalar.dma_start(out=ebufs[i][_ts(sc_, P), :], in_=zt[:])

        def ropeT(eng, ps, pool, span, outT):  # ps must be SBUF for gpsimd
            """RoPE in transposed (re,im)-permuted space; write fp8 outT.

            sinT carries [+sin, -sin] per 32-row half-block, so the
            swapped product lands pre-signed and every tensor_tensor has
            partition-aligned inputs (BIR requirement):
              out = ps*cos + swap32(ps)*sin_signed
            """
            sl = _ts(span, 512)
            tc_ = pool.tile([P, 512], F32, tag="rp_c")
            ts_ = pool.tile([P, 512], F32, tag="rp_s")
            eng.tensor_tensor(out=tc_[:], in0=ps, in1=cosT[:, sl], op=OP.mult)
            for h in range(NHL):
                re = slice(h * 64, h * 64 + 32)
                im = slice(h * 64 + 32, h * 64 + 64)
                eng.tensor_tensor(out=ts_[re, :], in0=ps[im, :],
                                  in1=sinT[im, sl], op=OP.mult)
                eng.tensor_tensor(out=ts_[im, :], in0=ps[re, :],
                                  in1=sinT[re, sl], op=OP.mult)
            eng.tensor_tensor(out=outT[:, sl], in0=tc_[:], in1=ts_[:],
                              op=OP.add)

        with tc.tile_pool(name="stA", bufs=3) as pa, \
             tc.tile_pool(name="stB", bufs=3) as pb, \
             tc.tile_pool(name="stC", bufs=3) as pc, \
             tc.tile_pool(name="st_ps", bufs=2, space="PSUM") as ps_pool, \
             tc.tile_pool(name="st_pst", bufs=2, space="PSUM") as pst_pool, \
             tc.tile_pool(name="st_av", bufs=1, space="PSUM") as av_pool:

            def projT(w_sb, span):
                ps = ps_pool.tile([P, 1024], F32, space="PSUM", tag="ps")
                w3 = w_sb[:].rearrange("p (j d) -> p j d", j=ND)
                for j in range(ND):
                    nc.tensor.matmul(
                        out=ps[:, 0:512], lhsT=w3[:, j, :],
                        rhs=xnT3[:, j, _ts(span, 512)],
                        start=(j == 0), stop=(j == ND - 1))
                return ps

            for b in range(B_FULL):
                for s in range(NSB):
                    ssl = _ts(s, 512)
                    # --- A: norm for the 4 token tiles of span s
                    xts, diags = [], []
                    for u in range(4):
                        tg = b * NTB + 4 * s + u
                        xt = pa.tile([P, DIM], BF, tag=f"xt{u}")
                        nc.sync.dma_start(out=xt[:], in_=xbf_in[_ts(tg, P), :])
                        sq = pa.tile([P, DIM], F32, tag="sq")
                        nc.gpsimd.tensor_tensor(out=sq[:], in0=xt[:],
                                                in1=xt[:], op=OP.mult)
                        ssq = pa.tile([P, 1], F32, tag="ssq")
                        nc.vector.reduce_sum(out=ssq[:], in_=sq[:], axis=AX.X)
                        rms = pa.tile([P, 1], F32, tag="rms")
                        nc.scalar.activation(out=rms[:], in_=ssq[:],
                                             func=ACTF.Sqrt,
                                             scale=1.0 / DIM, bias=eps_col[:])
                        rr = pa.tile([P, 1], F32, tag="rr")
                        nc.vector.reciprocal(out=rr[:], in_=rms[:])
                        diag = pa.tile([P, P], BF, tag=f"diag{u}")
                        nc.vector.tensor_scalar_mul(diag[:], id_bf[:], rr[:])
                        xts.append(xt)
                        diags.append(diag)
                    # transposes grouped per j: 4 tiles -> one [128,512] copy
                    for j in range(ND):
                        pst = pst_pool.tile([P, 512], BF, space="PSUM",
                                            tag="pstA")
                        for u in range(4):
                            nc.tensor.transpose(out=pst[:, _ts(u, P)],
                                                in_=xts[u][:, _ts(j, P)],
                                                identity=diags[u][:])
                        nc.vector.tensor_copy(out=xnT3[:, j, ssl], in_=pst[:])
                    # --- B: K/Q (rope'd, transposed) + V for span s
                    ps = projT(wk_sb, s)
                    ropeT(nc.vector, ps[:, 0:512], pb, s, kT)
                    ps = projT(wq_sb, s)
                    qf = pb.tile([P, 512], F32, tag="qf")
                    nc.scalar.activation(out=qf[:], in_=ps[:, 0:512],
                                         func=ACTF.Copy)
                    ropeT(nc.gpsimd, qf[:], pb, s, qT)
                    ps = projT(wv_sb, s)
                    vT = pb.tile([P, 512], BF, tag="vT")
                    nc.scalar.activation(out=vT[:], in_=ps[:, 0:512],
                                         func=ACTF.Copy)
                    pst = pst_pool.tile([P, 512], BF, space="PSUM", tag="pstA")
                    for u in range(4):
                        nc.tensor.transpose(out=pst[:, _ts(u, P)],
                                            in_=vT[:, _ts(u, P)],
                                            identity=id_bf[:])
                    # strided copy: psum (u h d) -> vaug (u [h d |1])
                    va4 = vaug[:, s * 4 * VW:(s + 1) * 4 * VW].rearrange(
                        "p (u h d) -> p u h d", u=4, h=NHL)
                    pst4 = pst[:].rearrange("p (u h d) -> p u h d", u=4, h=NHL)
                    nc.vector.tensor_copy(out=va4[:, :, :, 0:HD], in_=pst4)
                    nc.vector.memset(va4[:, :, :, HD:HD + 1], 1.0)

                # --- C: attention for batch b (2 heads, 2 query halves)
                for h in range(NHL):
                    hsl = slice(h * HD, (h + 1) * HD)
                    for qh in range(2):
                        aug = av_pool.tile([HD + 1, 1024], F32, space="PSUM",
                                           tag="aug")

                        def scores(kt):
                            sps = ps_pool.tile([P, 1024], F32, space="PSUM",
                                               tag="ps")
                            for u in range(2):
                                nc.tensor.matmul(
                                    out=sps[:, _ts(u, 512)],
                                    lhsT=kT[hsl, _ts(kt, P)],
                                    rhs=qT[hsl, qh * 1024 + u * 512:
                                           qh * 1024 + (u + 1) * 512],
                                    start=True, stop=True)
                            ex = pc.tile([P, 1024], BF, tag="expT")
                            nc.scalar.activation(out=ex[:], in_=sps[:],
                                                 func=ACTF.Exp)
                            return ex

                        def av(kt, ex):
                            t0 = kt * VW + h * (HD + 1)
                            for u in range(2):
                                nc.tensor.matmul(
                                    out=aug[:, _ts(u, 512)],
                                    lhsT=vaug[:, t0:t0 + HD + 1],
                                    rhs=ex[:, _ts(u, 512)],
                                    start=(kt == 0), stop=(kt == NTB - 1))

                        # software-pipelined: scores(kt+1) issued before
                        # av(kt) so the PE never stalls on the exp
                        prev = scores(0)
                        for kt in range(1, NTB):
                            cur = scores(kt)
                            av(kt - 1, prev)
                            prev = cur
                        av(NTB - 1, prev)
                        rcp = pc.tile([1, 1024], F32, tag="rcp")
                        nc.vector.reciprocal(out=rcp[:], in_=aug[HD:HD + 1, :])
                        rbc = pc.tile([HD, 1024], F32, tag="rbc")
                        nc.gpsimd.partition_broadcast(rbc[:], rcp[:])
                        nc.vector.tensor_tensor(
                            out=attT_sb[hsl, b * S_FULL + qh * 1024:
                                        b * S_FULL + qh * 1024 + 1024],
                            in0=aug[0:HD, :], in1=rbc[:], op=OP.mult)
        sc_xnT.close()
        nc.leave_named_scope("ABC_attn", scAB[0], False)

        # =============== attention AllToAll ==========================
        scA2A = nc.enter_named_scope("A2A_att", False)
        for c in range(n_cores):
            nc.sync.dma_start(out=att_loc.ap()[_ts(c, P), :],
                              in_=attT_sb[:, _ts(c, LQ)])
        nc.gpsimd.collective_compute(
            "AllToAll", OP.bypass, replica_groups=all_groups,
            ins=[att_loc.ap().opt()], outs=[att_recv.ap().opt()])
        attd = p_att.tile([P, ND * LQ], BF, name="attd", tag="attd")
        attd3 = attd[:].rearrange("p (j t) -> p j t", j=ND)
        for j in range(ND):
            nc.sync.dma_start(out=attd3[:, j, :], in_=att_recv[_ts(j, P), :])
        sc_kv.close()
        nc.leave_named_scope("A2A_att", scA2A[0], False)

        def load_big(pool, src, tag, cols, dt=E4):
            w = pool.tile([P, ND * cols], dt, name=tag, tag=tag)
            nc.sync.dma_start(out=w[:], in_=src[:, :])
            return w[:].rearrange("p (j c) -> p j c", j=ND)

        # =============== stage D: O-proj, gate, hn ===================
        scD = nc.enter_named_scope("D_oproj", False)
        with tc.tile_pool(name="stD", bufs=3) as pd, \
             tc.tile_pool(name="stD_w", bufs=1) as pdw, \
             tc.tile_pool(name="stD_ps", bufs=2, space="PSUM") as pd_ps, \
             tc.tile_pool(name="stD_gps", bufs=2, space="PSUM") as pd_gps, \
             tc.tile_pool(name="stD_pst", bufs=2, space="PSUM") as pd_pst, \
             tc.tile_pool(name="stD_hT", bufs=1) as pd_hT:
            wo3 = load_big(pdw, wo_in, "wo", DIM, dt=BF)
            gate_sb = pdw.tile([P, ND * E], F32, name="g32", tag="g32")
            nc.sync.dma_start(out=gate_sb[:], in_=gate_in[:, :])
            gate3 = gate_sb[:].rearrange("p (j e) -> p j e", j=ND)
            hT32 = [pd_hT.tile([P, LQ], F32, name=f"hT{j}", tag=f"hT{j}")
                    for j in range(ND)]
            hnT3 = hnT[:].rearrange("p (j t) -> p j t", j=ND)
            for t in range(NQ):
                ps = pd_ps.tile([P, DIM], F32, space="PSUM", tag="ops")
                for half in range(2):
                    for j in range(ND):
                        nc.tensor.matmul(
                            out=ps[:, _ts(half, 512)],
                            lhsT=attd3[:, j, _ts(t, P)],
                            rhs=wo3[:, j, _ts(half, 512)],
                            start=(j == 0), stop=(j == ND - 1))
                xres = pd.tile([P, DIM], F32, tag="xres")
                nc.sync.dma_start(out=xres[:], in_=x_in[_ts(t, P), :])
                nc.vector.tensor_tensor(out=h_sb[t][:], in0=ps[:],
                                        in1=xres[:], op=OP.add)
                for j in range(ND):
                    pst = pd_pst.tile([P, P], F32, space="PSUM", tag="pstD")
                    nc.tensor.transpose(out=pst[:], in_=h_sb[t][:, _ts(j, P)],
                                        identity=id_f32[:])
                    nc.vector.tensor_copy(out=hT32[j][:, _ts(t, P)], in_=pst[:])
            # fp32 gate logits first (so the tiny logits AllGather can
            # fire early and routing overlaps the hn AllGather)
            for t in range(NQ):
                gps = pd_gps.tile([P, E], F32, space="PSUM", tag="gps")
                for j in range(ND):
                    nc.tensor.matmul(out=gps[:], lhsT=hT32[j][:, _ts(t, P)],
                                     rhs=gate3[:, j, :],
                                     start=(j == 0), stop=(j == ND - 1))
                sq = pd.tile([P, DIM], F32, tag="sqD")
                ssq = pd.tile([P, 1], F32, tag="ssqD")
                nc.scalar.activation(out=sq[:], in_=h_sb[t][:], func=ACTF.Square,
                                     accum_out=ssq[:])
                rms = pd.tile([P, 1], F32, tag="rmsD")
                nc.scalar.activation(out=rms[:], in_=ssq[:], func=ACTF.Sqrt,
                                     scale=1.0 / DIM, bias=eps_col[:])
                nc.vector.reciprocal(out=rr_sb[t][:], in_=rms[:])
                lg = pd.tile([P, E], F32, tag="lg")
                nc.vector.tensor_scalar_mul(lg[:], gps[:], rr_sb[t][:])
                nc.sync.dma_start(out=lg_loc[_ts(t, P), :], in_=lg[:])
            nc.gpsimd.collective_compute(
                "AllGather", OP.bypass, replica_groups=all_groups,
                ins=[lg_loc.ap().opt()], outs=[lg_full.ap().opt()])
            for t in range(NQ):
                hn = pd.tile([P, DIM], BF, tag="hnD")
                nc.scalar.activation(out=hn[:], in_=h_sb[t][:], func=ACTF.Copy,
                                     scale=rr_sb[t][:])
                nc.sync.dma_start(out=hn_loc[_ts(t, P), :], in_=hn[:])
                for j in range(ND):
                    pst = pd_pst.tile([P, P], BF, space="PSUM", tag="pstD")
                    nc.tensor.transpose(out=pst[:], in_=hn[:, _ts(j, P)],
                                        identity=id_bf[:])
                    nc.vector.tensor_copy(out=hnT3[:, j, _ts(t, P)], in_=pst[:])
        sc_att.close()
        nc.leave_named_scope("D_oproj", scD[0], False)

        # =============== hn AllGather ================================
        scCC = nc.enter_named_scope("CC_gather", False)
        nc.gpsimd.collective_compute(
            "AllGather", OP.bypass, replica_groups=all_groups,
            ins=[hn_loc.ap().opt()], outs=[hn_full.ap().opt()])
        nc.leave_named_scope("CC_gather", scCC[0], False)

        # =============== stage H: shared expert + local output =======
        # (independent of the gathers: its TensorE work fills the window)
        scH = nc.enter_named_scope("H_shared", False)
        with tc.tile_pool(name="stH", bufs=3) as ph, \
             tc.tile_pool(name="stH_w", bufs=1) as phw, \
             tc.tile_pool(name="stH_gT", bufs=1) as ph_gT, \
             tc.tile_pool(name="stH_ps", bufs=2, space="PSUM") as ph_ps:
            s1_3 = load_big(phw, sw1_in, "s1", HID)
            s3_3 = load_big(phw, sw3_in, "s3", HID)
            gsT = ph_gT.tile([P, ND * LQ], E4, name="gsT", tag="gsT")
            gsT3 = gsT[:].rearrange("p (j t) -> p j t", j=ND)
            hnT3 = hnT[:].rearrange("p (j t) -> p j t", j=ND)
            for j in range(ND):
                h1 = ph_ps.tile([P, LQ], F32, space="PSUM", tag="sh1")
                h3 = ph_ps.tile([P, LQ], F32, space="PSUM", tag="sh3")
                for d in range(ND // 2):
                    nc.tensor.matmul(out=h1[:],
                                     lhsT=s1_3[:, 2 * d:2 * d + 2, _ts(j, P)],
                                     rhs=hnT3[:, 2 * d:2 * d + 2, :],
                                     start=(d == 0), stop=(d == ND // 2 - 1),
                                     perf_mode=DR)
                for d in range(ND // 2):
                    nc.tensor.matmul(out=h3[:],
                                     lhsT=s3_3[:, 2 * d:2 * d + 2, _ts(j, P)],
                                     rhs=hnT3[:, 2 * d:2 * d + 2, :],
                                     start=(d == 0), stop=(d == ND // 2 - 1),
                                     perf_mode=DR)
                sig = ph.tile([P, LQ], F32, tag="sigH")
                nc.scalar.activation(out=sig[:], in_=h1[:], func=ACTF.Sigmoid,
                                     scale=1.0 / SW)
                nc.vector.tensor_tensor(out=sig[:], in0=sig[:], in1=h1[:],
                                        op=OP.mult)
                nc.vector.scalar_tensor_tensor(
                    out=gsT3[:, j, :], in0=sig[:], scalar=1.0 / (SW * SW),
                    in1=h3[:], op0=OP.mult, op1=OP.mult)
            s2_3 = load_big(phw, sw2_in, "s2", DIM)
            for t in range(NQ):
                ps = ph_ps.tile([P, DIM], F32, space="PSUM", tag="shps")
                for half in range(2):
                    for j in range(ND // 2):
                        nc.tensor.matmul(
                            out=ps[:, _ts(half, 512)],
                            lhsT=gsT3[:, 2 * j:2 * j + 2, _ts(t, P)],
                            rhs=s2_3[:, 2 * j:2 * j + 2, _ts(half, 512)],
                            start=(j == 0), stop=(j == ND // 2 - 1),
                            perf_mode=DR)
                ot = ph.tile([P, DIM], F32, tag="ot")
                nc.vector.scalar_tensor_tensor(
                    out=ot[:], in0=ps[:], scalar=1.0 / SW,
                    in1=h_sb[t][:], op0=OP.mult, op1=OP.add)
                nc.sync.dma_start(out=out_local[_ts(t, P), :], in_=ot[:])
        nc.leave_named_scope("H_shared", scH[0], False)

        # =============== stage F: routing + dispatch =================
        # Selection on raw fp32 logits (host replicates it from lg_out).
        scF = nc.enter_named_scope("F_route", False)
        with tc.tile_pool(name="stF", bufs=8) as pf, \
             tc.tile_pool(name="stF_keep", bufs=1) as pfk, \
             tc.tile_pool(name="stF_ps", bufs=2, space="PSUM") as pf_ps, \
             tc.tile_pool(name="stF_tot", bufs=1, space="PSUM") as pf_tot:
            lg_all = pfk.tile([P, NA * E], F32)
            nc.sync.dma_start(
                out=lg_all[:].rearrange("p (t e) -> p t e", t=NA),
                in_=lg_full.ap().rearrange("(t p) e -> p t e", p=P))
            nc.sync.dma_start(
                out=lg_out.ap().rearrange("(t p) e -> p t e", p=P),
                in_=lg_all[:].rearrange("p (t e) -> p t e", t=NA))
            v3 = lg_all[:].rearrange("p (t e) -> p t e", t=NA)
            m1 = pfk.tile([P, NA], F32)
            nc.vector.reduce_max(out=m1[:], in_=v3, axis=AX.X)
            ge1 = pfk.tile([P, NA * E], F32)
            g13 = ge1[:].rearrange("p (t e) -> p t e", t=NA)
            nc.vector.tensor_tensor(out=g13, in0=v3,
                                    in1=m1[:, :, None].to_broadcast([P, NA, E]),
                                    op=OP.is_ge)
            msk = pfk.tile([P, NA * E], F32)
            nc.vector.tensor_scalar_mul(msk[:], ge1[:], -1.0e30)
            nc.vector.tensor_tensor(out=msk[:], in0=msk[:], in1=lg_all[:],
                                    op=OP.add)
            m2 = pfk.tile([P, NA], F32)
            nc.vector.reduce_max(out=m2[:],
                                 in_=msk[:].rearrange("p (t e) -> p t e", t=NA),
                                 axis=AX.X)
            ge = pfk.tile([P, NA * E], F32)
            ge3 = ge[:].rearrange("p (t e) -> p t e", t=NA)
            nc.vector.tensor_tensor(out=ge3, in0=v3,
                                    in1=m2[:, :, None].to_broadcast([P, NA, E]),
                                    op=OP.is_ge)
            msel = pfk.tile([P, NA * E], F32)
            nc.vector.tensor_tensor(out=msel[:], in0=ge[:], in1=oh_bc[:],
                                    op=OP.mult)
            ind = pfk.tile([P, NA], F32)
            nc.vector.reduce_sum(out=ind[:],
                                 in_=msel[:].rearrange("p (t e) -> p t e", t=NA),
                                 axis=AX.X)
            # per-tile totals + within-tile inclusive cumsum: one matmul each
            tots = pf_tot.tile([1, NA], F32, space="PSUM")
            nc.tensor.matmul(out=tots[:], lhsT=ones_col[:], rhs=ind[:],
                             start=True, stop=True)
            cnts = pf_tot.tile([P, NA], F32, space="PSUM")
            nc.tensor.matmul(out=cnts[:], lhsT=ltri[:], rhs=ind[:],
                             start=True, stop=True)
            # batched exclusive cumsum of tile totals -> per-tile bases
            tots_sb = pf.tile([1, NA], F32, tag="tots_sb")
            nc.vector.tensor_copy(out=tots_sb[:], in_=tots[:])
            totsT_ps = pf_ps.tile([NA, 1], F32, space="PSUM", tag="totsT", bufs=1)
            nc.tensor.transpose(out=totsT_ps[:], in_=tots_sb[:],
                                identity=id_f32[:1, :1])
            totsT = pf.tile([NA, 1], F32, tag="totsT_sb")
            nc.vector.tensor_copy(out=totsT[:], in_=totsT_ps[:])
            basesT_ps = pf_ps.tile([NA, 1], F32, space="PSUM", tag="basesT", bufs=1)
            nc.tensor.matmul(out=basesT_ps[:], lhsT=ltri_s[:NA, :NA],
                             rhs=totsT[:], start=True, stop=True)
            basesT = pf.tile([NA, 1], F32, tag="basesT_sb")
            nc.vector.tensor_copy(out=basesT[:], in_=basesT_ps[:])
            bases_ps = pf_ps.tile([1, NA], F32, space="PSUM", tag="bases", bufs=1)
            nc.tensor.transpose(out=bases_ps[:], in_=basesT[:],
                                identity=id_f32[:NA, :NA])
            bases_sb = pf.tile([1, NA], F32, tag="bases_sb")
            nc.vector.tensor_copy(out=bases_sb[:], in_=bases_ps[:])
            bb_all = pfk.tile([P, NA], F32)
            nc.gpsimd.partition_broadcast(bb_all[:], bases_sb[:])
            # destinations (batched)
            d_all = pfk.tile([P, NA], F32)
            nc.vector.scalar_tensor_tensor(
                out=d_all[:], in0=cnts[:], scalar=-(1.0 + BIG),
                in1=bb_all[:], op0=OP.add, op1=OP.add)
            nc.vector.tensor_tensor(out=d_all[:], in0=d_all[:], in1=ind[:],
                                    op=OP.mult)
            nc.vector.tensor_scalar_add(d_all[:], d_all[:], BIG)
            dest_all = pfk.tile([P, NA], I32)
            nc.vector.tensor_copy(out=dest_all[:], in_=d_all[:])
            # scatters (independent per tile)
            for t in range(NA):
                hnt = pf.tile([P, DIM], BF, tag="hnF")
                nc.sync.dma_start(out=hnt[:], in_=hn_full[_ts(t, P), :])
                nc.gpsimd.indirect_dma_start(
                    out=ebufs[t % NSPLIT][:, :],
                    out_offset=bass.IndirectOffsetOnAxis(
                        ap=dest_all[:, t:t + 1], axis=0),
                    in_=hnt[:], in_offset=None,
                    bounds_check=C - 1, oob_is_err=False)
        nc.leave_named_scope("F_route", scF[0], False)

        # =============== stage G: expert FFN =========================
        scG = nc.enter_named_scope("G_expert", False)
        with tc.tile_pool(name="stG", bufs=3) as pg, \
             tc.tile_pool(name="stG_w", bufs=1) as pgw, \
             tc.tile_pool(name="stG_gT", bufs=1) as pg_gT:
            ebT = pg_gT.tile([P, ND * C], E4, name="ebT", tag="ebT")
            ebT3 = ebT[:].rearrange("p (j t) -> p j t", j=ND)
            with tc.tile_pool(name="stG_ps", bufs=4, space="PSUM") as pg_ps:
                for s in range(NCAP):
                    parts = []
                    for i in range(NSPLIT):
                        pt = pg.tile([P, DIM], BF, tag=f"ebp{i}", bufs=2)
                        nc.sync.dma_start(out=pt[:], in_=ebufs[i][_ts(s, P), :])
                        parts.append(pt)
                    nc.vector.tensor_tensor(out=parts[0][:], in0=parts[0][:],
                                            in1=parts[1][:], op=OP.add)
                    nc.vector.tensor_tensor(out=parts[2][:], in0=parts[2][:],
                                            in1=parts[3][:], op=OP.add)
                    eb = pg.tile([P, DIM], BF, tag="eb")
                    nc.vector.tensor_tensor(out=eb[:], in0=parts[0][:],
                                            in1=parts[2][:], op=OP.add)
                    for j in range(ND):
                        pst = pg_ps.tile([P, P], BF, space="PSUM", tag="pstG")
                        nc.tensor.transpose(out=pst[:], in_=eb[:, _ts(j, P)],
                                            identity=id_bf[:])
                        nc.vector.tensor_copy(out=ebT3[:, j, _ts(s, P)],
                                              in_=pst[:])
            e1_3 = load_big(pgw, ew1_in, "e1", HID)
            e3_3 = load_big(pgw, ew3_in, "e3", HID)
            gT = pg_gT.tile([P, ND * C], E4, name="gT", tag="gT")
            gT3 = gT[:].rearrange("p (j t) -> p j t", j=ND)
            nsub = (C + 511) // 512
            with tc.tile_pool(name="stG_ps2", bufs=2, space="PSUM") as pg_ps2:
                for j in range(ND):
                    for s in range(nsub):
                        w = min(512, C - s * 512)
                        sl = slice(s * 512, s * 512 + w)
                        h1 = pg_ps2.tile([P, 512], F32, space="PSUM", tag="h1")
                        h3 = pg_ps2.tile([P, 512], F32, space="PSUM", tag="h3")
                        for d in range(ND // 2):
                            nc.tensor.matmul(
                                out=h1[:, :w],
                                lhsT=e1_3[:, 2 * d:2 * d + 2, _ts(j, P)],
                                rhs=ebT3[:, 2 * d:2 * d + 2, sl],
                                start=(d == 0), stop=(d == ND // 2 - 1),
                                perf_mode=DR)
                        for d in range(ND // 2):
                            nc.tensor.matmul(
                                out=h3[:, :w],
                                lhsT=e3_3[:, 2 * d:2 * d + 2, _ts(j, P)],
                                rhs=ebT3[:, 2 * d:2 * d + 2, sl],
                                start=(d == 0), stop=(d == ND // 2 - 1),
                                perf_mode=DR)
                        sig = pg.tile([P, 512], F32, tag="sig")
                        nc.scalar.activation(out=sig[:, :w], in_=h1[:, :w],
                                             func=ACTF.Sigmoid, scale=1.0 / SW)
                        nc.vector.tensor_tensor(out=sig[:, :w], in0=sig[:, :w],
                                                in1=h1[:, :w], op=OP.mult)
                        nc.vector.scalar_tensor_tensor(
                            out=gT3[:, j, sl], in0=sig[:, :w],
                            scalar=1.0 / (SW * SW), in1=h3[:, :w],
                            op0=OP.mult, op1=OP.mult)
                e2_3 = load_big(pgw, ew2_in, "e2", DIM)
                for s in range(NCAP):
                    ps = pg_ps2.tile([P, DIM], F32, space="PSUM", tag="eops")
                    for half in range(2):
                        for j in range(ND // 2):
                            nc.tensor.matmul(
                                out=ps[:, _ts(half, 512)],
                                lhsT=gT3[:, 2 * j:2 * j + 2, _ts(s, P)],
                                rhs=e2_3[:, 2 * j:2 * j + 2, _ts(half, 512)],
                                start=(j == 0), stop=(j == ND // 2 - 1),
                                perf_mode=DR)
                    eo = pg.tile([P, DIM], F32, tag="eo")
                    nc.scalar.activation(out=eo[:], in_=ps[:], func=ACTF.Copy,
                                         scale=1.0 / SW)
                    nc.sync.dma_start(out=eo_out[_ts(s, P), :], in_=eo[:])
        nc.leave_named_scope("G_expert", scG[0], False)

    nc.compile()
    return nc


# ----------------------------------------------------------------------
# host side
# ----------------------------------------------------------------------

def _tile128(w):
    """[R, C] -> [128, (R/128)*C] with row-tile-major layout."""
    R, Cc = w.shape
    return np.ascontiguousarray(
        w.reshape(R // P, P, Cc).transpose(1, 0, 2).reshape(P, (R // P) * Cc))


def prep_inputs(x, freqs, att_norm_w, wq, wk, wv, wo, ffn_norm_w, gate_w,
                ew1, ew2, ew3, sw1, sw2, sw3, LQ=LQ_FULL, n_cores=8):
    """Build the 8 per-core input maps (host-side weight folding + slicing)."""
    def to8(a):
        return np.ascontiguousarray(np.asarray(a, np.float32).astype(FP8))

    def tobf(a):
        return np.ascontiguousarray(np.asarray(a, np.float32).astype(BF16))

    B, S, _ = x.shape
    N = B * S
    anw = np.asarray(att_norm_w, np.float32)
    fnw = np.asarray(ffn_norm_w, np.float32)
    wq_n = (anw[:, None] * np.asarray(wq, np.float32)) / np.sqrt(HD)
    wk_n = anw[:, None] * np.asarray(wk, np.float32)
    wv_n = anw[:, None] * np.asarray(wv, np.float32)
    wo_e = tobf(_tile128(np.asarray(wo, np.float32)))
    gate32 = np.ascontiguousarray(
        _tile128((np.asarray(gate_w, np.float32) * fnw[None, :]).T))
    ew1_e = np.asarray(ew1, np.float32) * fnw[None, :, None] * SW
    ew3_e = np.asarray(ew3, np.float32) * fnw[None, :, None] * SW
    ew2_e = np.asarray(ew2, np.float32) * SW
    sw1_e = to8(_tile128(np.asarray(sw1, np.float32) * fnw[:, None] * SW))
    sw3_e = to8(_tile128(np.asarray(sw3, np.float32) * fnw[:, None] * SW))
    sw2_e = to8(_tile128(np.asarray(sw2, np.float32) * SW))

    x_flat = np.asarray(x, np.float32).reshape(N, DIM)
    x_bf = np.ascontiguousarray(x_flat.astype(BF16))
    # rope tables in transposed space: row r -> pair (r % 32)
    cos32 = np.asarray(freqs[:S, :, 0], np.float32).T        # (32, S)
    sin32 = np.asarray(freqs[:S, :, 1], np.float32).T
    cosT = np.ascontiguousarray(np.tile(cos32, (4, 1)))      # (128, S)
    # signed sin: +sin on re rows (0-31 of each head block), -sin on im rows
    sinT = np.ascontiguousarray(np.tile(np.vstack([sin32, -sin32]), (2, 1)))
    # within-head (re, im) column permutation for transposed-space rope
    pidx = np.concatenate([np.arange(0, HD, 2), np.arange(1, HD, 2)])

    in_maps = []
    for core in range(n_cores):
        heads = [2 * core, 2 * core + 1]
        wq_c = np.hstack([wq_n[:, h * HD + pidx] for h in heads])
        wk_c = np.hstack([wk_n[:, h * HD + pidx] for h in heads])
        wv_c = np.hstack([wv_n[:, _ts(h, HD)] for h in heads])
        na = N // 128
        oh = np.zeros((1, E), np.float32)
        oh[0, core % E] = 1.0
        oh = np.tile(oh, (1, na))
        in_maps.append(dict(
            x_bf=x_bf,
            x_chunk=np.ascontiguousarray(x_flat[core * LQ:(core + 1) * LQ]),
            cosT=cosT, sinT=sinT,
            wq_t8=tobf(_tile128(wq_c)), wk_t8=tobf(_tile128(wk_c)),
            wv_t8=tobf(_tile128(wv_c)),
            wo_t8=wo_e, gate_t32=gate32,
            sw1_t8=sw1_e, sw2_t8=sw2_e, sw3_t8=sw3_e,
            ew1_t8=to8(_tile128(ew1_e[core % E])),
            ew2_t8=to8(_tile128(ew2_e[core % E])),
            ew3_t8=to8(_tile128(ew3_e[core % E])),
            onehot=oh,
        ))
    return in_maps


def assemble(results, B, S, LQ=LQ_FULL, n_cores=8):
    N = B * S
    out = np.zeros((N, DIM), np.float32)
    y = np.zeros((N, DIM), np.float32)
    # replicate the device's top-2 selection exactly from the fp32 logits
    lg = np.asarray(results[0]["lg_out"], np.float32)          # (N, E)
    m2 = np.partition(lg, -2, axis=1)[:, -2]
    sel_mask = lg >= m2[:, None]
    ex = np.exp(lg - lg.max(axis=1, keepdims=True), dtype=np.float32)
    probs = ex / ex.sum(axis=1, keepdims=True, dtype=np.float32)
    for core, res in enumerate(results):
        tok0 = core * LQ
        out[tok0:tok0 + LQ] = res["out_local"]
        e = core % E
        sel = np.nonzero(sel_mask[:, e])[0]
        cnt = len(sel)
        eo = res["eo_out"]
        assert cnt <= eo.shape[0], (core, cnt)
        y[sel] += probs[sel, e:e + 1] * eo[:cnt]
    return (out + y).reshape(B, S, DIM)


_NC_CACHE = {}


def kernel(**inputs):
    key = "full"
    if key not in _NC_CACHE:
        _NC_CACHE[key] = build_nc()
    nc = _NC_CACHE[key]
    from concourse.bass_utils import run_bass_kernel_spmd
    in_maps = prep_inputs(**inputs)
    res = run_bass_kernel_spmd(nc, in_maps, core_ids=list(range(8)))
    x = np.asarray(inputs["x"])
    return assemble(res.results, x.shape[0], x.shape[1]).astype(np.float32)


if __name__ == "__main__":
    nc = build_nc()
    print("built + compiled OK")


# revision 42
# speedup vs baseline: 1.1918x; 1.0850x over previous
"""Trainium2 Bass kernel for nn_Block_24343874633736 (moe_routing).

Transformer block: RMSNorm -> MHA(RoPE) -> residual -> RMSNorm ->
MoE (8 routed experts, top-2, + 1 shared expert) -> residual.

Sharding (8 NeuronCores, single SPMD launch):
  - Attention is HEAD-sharded: every core normalizes all 4096 tokens
    and computes K/V/Q + attention for its 2 of 16 heads over both
    batches (processed batch-by-batch to halve SBUF residency). One
    8-core AllToAll then swaps (head-dims x token-chunks) so each core
    ends with all 16 heads' attention output for its 512 local tokens.
    No K/V collectives; the initial collective barrier hides under
    compute.
  - K/Q are computed directly in transposed layout (weights stationary,
    tokens on the free axis) with RoPE applied in transposed space via
    a per-head (re-pairs, im-pairs) weight-column permutation and a
    sign-folded sin table; rope is split across Vector (q) and
    GpSimd (k).
  - MoE: expert-parallel, one routed expert per core. A tiny fp32
    logits AllGather fires first so routing overlaps the bf16 hn
    AllGather; the shared expert fills the gather window. Compaction
    uses triangular-matmul cumsums + indirect-DMA scatters into NSPLIT
    DRAM buffers; the host scatter-adds.

Numerics: everything upstream of the gate logits (norm, Q/K/V,
attention, O-proj) is bf16 with fp32 PSUM so the fp32 gate logits stay
close to the reference and near-tie top-2 flips stay rare (each flip
is a large localized error). The y-path (shared + routed expert FFNs),
which cannot flip routing, runs fp8e4m3 with DoubleRow (0.5 cyc/row);
those weights are pre-scaled x8 host-side to clear the fp8 denormal
range, with descales folded into activation-scale parameters. The host
replicates top-2 selection exactly from lg_out.
"""

import sys

for _p in ("/opt/trn_rl_repo",):
    if _p not in sys.path:
        sys.path.insert(0, _p)

import numpy as np
import ml_dtypes

import concourse.bass as bass
import concourse.mybir as mybir
from concourse import bacc
from concourse.masks import make_identity, make_upper_triangular
from concourse.tile import TileContext

BF16 = ml_dtypes.bfloat16
FP8 = ml_dtypes.float8_e4m3
F32 = mybir.dt.float32
BF = mybir.dt.bfloat16
E4 = mybir.dt.float8e4
I32 = mybir.dt.int32
AX = mybir.AxisListType
OP = mybir.AluOpType
ACTF = mybir.ActivationFunctionType
DR = mybir.MatmulPerfMode.DoubleRow

P = 128
DIM = 1024
NH = 16
HD = 64
E = 8
HID = 1024
EPS = 1e-6
BIG = 60000.0  # trash slot index (> any capacity; exact in fp32/int32)

B_FULL, S_FULL = 2, 2048
NTOK = B_FULL * S_FULL    # 4096 tokens total
LQ_FULL = 512             # tokens owned per core
C_FULL = 1152             # per-expert token capacity (fp64 max count 1062)
NHL = 2                   # heads per core

# fp8 weight pre-scale for the expert FFNs (clears the denormal range)
SW = 8.0


def _ts(i, n):
    return slice(i * n, (i + 1) * n)


def build_nc(LQ=LQ_FULL, C=C_FULL, n_cores=8):
    """Emit the SPMD Bass program. All 8 cores run this same program."""
    NSB = S_FULL // 512      # 4 projection spans per batch
    NTB = S_FULL // P        # 16 token tiles per batch
    NQ = LQ // P             # 4 local tiles
    NCAP = C // P
    ND = DIM // P
    NA = NTOK // P           # 32
    assert NA <= P

    nc = bacc.Bacc("TRN2", target_bir_lowering=False, debug=False,
                   num_devices=n_cores)

    # ---- I/O (weights arrive 128-row pre-tiled: [P, ntiles*cols]) ----
    xbf_in = nc.dram_tensor("x_bf", [NTOK, DIM], BF, kind="ExternalInput")
    x_in = nc.dram_tensor("x_chunk", [LQ, DIM], F32, kind="ExternalInput")
    cosT_in = nc.dram_tensor("cosT", [P, S_FULL], F32, kind="ExternalInput")
    sinT_in = nc.dram_tensor("sinT", [P, S_FULL], F32, kind="ExternalInput")
    wq_in = nc.dram_tensor("wq_t8", [P, DIM], BF, kind="ExternalInput")
    wk_in = nc.dram_tensor("wk_t8", [P, DIM], BF, kind="ExternalInput")
    wv_in = nc.dram_tensor("wv_t8", [P, DIM], BF, kind="ExternalInput")
    wo_in = nc.dram_tensor("wo_t8", [P, ND * DIM], BF, kind="ExternalInput")
    gate_in = nc.dram_tensor("gate_t32", [P, ND * E], F32, kind="ExternalInput")
    sw1_in = nc.dram_tensor("sw1_t8", [P, ND * HID], BF, kind="ExternalInput")
    sw2_in = nc.dram_tensor("sw2_t8", [P, ND * DIM], BF, kind="ExternalInput")
    sw3_in = nc.dram_tensor("sw3_t8", [P, ND * HID], BF, kind="ExternalInput")
    ew1_in = nc.dram_tensor("ew1_t8", [P, ND * HID], E4, kind="ExternalInput")
    ew2_in = nc.dram_tensor("ew2_t8", [P, ND * DIM], E4, kind="ExternalInput")
    ew3_in = nc.dram_tensor("ew3_t8", [P, ND * HID], E4, kind="ExternalInput")
    oh_in = nc.dram_tensor("onehot", [1, NA * E], F32, kind="ExternalInput")

    out_local = nc.dram_tensor("out_local", [LQ, DIM], F32, kind="ExternalOutput")
    eo_out = nc.dram_tensor("eo_out", [C, DIM], F32, kind="ExternalOutput")
    lg_out = nc.dram_tensor("lg_out", [NTOK, E], F32, kind="ExternalOutput")

    # internal DRAM
    warm_in = nc.dram_tensor("warm_in", [8, 64], BF)
    warm_out = nc.dram_tensor("warm_out", [8, 64], BF)
    att_loc = nc.dram_tensor("att_loc", [n_cores * P, LQ], BF)
    att_recv = nc.dram_tensor("att_recv", [n_cores * P, LQ], BF)
    lg_loc = nc.dram_tensor("lg_loc", [LQ, E], F32)
    lg_full = nc.dram_tensor("lg_full", [NTOK, E], F32, addr_space="Shared")
    hn_loc = nc.dram_tensor("hn_loc", [LQ, DIM], BF)
    hn_full = nc.dram_tensor("hn_full", [NTOK, DIM], BF, addr_space="Shared")
    NSPLIT = 4  # scatter-chain split: breaks the WAW serialization
    ebufs = [nc.dram_tensor(f"ebuf{i}", [C, DIM], BF) for i in range(NSPLIT)]

    all_groups = [list(range(n_cores))]

    def r3(tile, n, width):
        return tile[:].rearrange("p (j c) -> p j c", j=n)

    from contextlib import ExitStack
    with TileContext(nc) as tc, ExitStack() as stack:
        const_pool = stack.enter_context(tc.tile_pool(name="const", bufs=1))
        id_bf = const_pool.tile([P, P], BF)
        make_identity(nc, id_bf[:])
        id_f32 = const_pool.tile([P, P], F32)
        make_identity(nc, id_f32[:])
        ltri = const_pool.tile([P, P], F32)
        make_upper_triangular(nc, ltri[:], val=1.0, diag=True)  # L[k,p]=1 iff k<=p
        ltri_s = const_pool.tile([P, P], F32)
        make_upper_triangular(nc, ltri_s[:], val=1.0, diag=False)  # k<p
        ones_col = const_pool.tile([P, 1], F32)
        nc.vector.memset(ones_col[:], 1.0)
        eps_col = const_pool.tile([P, 1], F32)
        nc.vector.memset(eps_col[:], EPS)
        oh_bc = const_pool.tile([P, NA * E], F32)
        oh_row = const_pool.tile([1, NA * E], F32)
        nc.sync.dma_start(out=oh_row[:], in_=oh_in[:, :])
        nc.gpsimd.partition_broadcast(oh_bc[:], oh_row[:])

        # warmup AllToAll: pays the cold collective-stream cost under
        # compute so the real att AllToAll later runs warm
        wtile = const_pool.tile([8, 64], BF)
        nc.vector.memset(wtile[:], 0.0)
        nc.sync.dma_start(out=warm_in[:, :], in_=wtile[:])
        nc.gpsimd.collective_compute(
            "AllToAll", OP.bypass, replica_groups=all_groups,
            ins=[warm_in.ap().opt()], outs=[warm_out.ap().opt()])

        # persistent activations
        persist = stack.enter_context(tc.tile_pool(name="persist", bufs=1))
        h_sb = [persist.tile([P, DIM], F32, name=f"h{i}", tag=f"h{i}")
                for i in range(NQ)]
        hnT = persist.tile([P, ND * LQ], BF, name="hnT", tag="hnT")
        rr_sb = [persist.tile([P, 1], F32, name=f"rr{i}", tag=f"rr{i}")
                 for i in range(NQ)]

        # LIFO-nested scoped pools
        sc_att = ExitStack()   # attd (post-A2A), closes after D
        p_att = sc_att.enter_context(tc.tile_pool(name="p_att", bufs=1))
        sc_kv = ExitStack()    # kT/qT/vaug/attT_sb, closes after A2A send
        p_kv = sc_kv.enter_context(tc.tile_pool(name="p_kv", bufs=1))
        sc_xnT = ExitStack()   # xnT + rope tables + w slices, closes after B
        p_xnT = sc_xnT.enter_context(tc.tile_pool(name="p_xnT", bufs=1))

        # =============== stages A/B/C: norm, K/V/Q, attention ========
        scAB = nc.enter_named_scope("ABC_attn", False)
        xnT = p_xnT.tile([P, ND * S_FULL], BF, name="xnT", tag="xnT")
        xnT3 = xnT[:].rearrange("p (j t) -> p j t", j=ND)
        cosT = p_xnT.tile([P, S_FULL], F32, name="cosT", tag="cosT")
        sinT = p_xnT.tile([P, S_FULL], F32, name="sinT", tag="sinT")
        nc.sync.dma_start(out=cosT[:], in_=cosT_in[:, :])
        nc.sync.dma_start(out=sinT[:], in_=sinT_in[:, :])
        wk_sb = p_xnT.tile([P, DIM], BF, name="wk", tag="wk")
        wq_sb = p_xnT.tile([P, DIM], BF, name="wq", tag="wq")
        wv_sb = p_xnT.tile([P, DIM], BF, name="wv", tag="wv")
        for w_sb, w_in in ((wk_sb, wk_in), (wq_sb, wq_in), (wv_sb, wv_in)):
            nc.scalar.dma_start(out=w_sb[:], in_=w_in[:, :])

        kT = p_kv.tile([P, S_FULL], BF, name="kT", tag="kT")
        qT = p_kv.tile([P, S_FULL], BF, name="qT", tag="qT")
        VW = NHL * (HD + 1)  # 130 cols per token tile: (64 v + 1 one) x 2 heads
        vaug = p_kv.tile([P, NTB * VW], BF, name="vaug", tag="vaug")
        attT_sb = p_kv.tile([P, NTOK], BF, name="attT", tag="attT")

        # zero the ebuf split buffers early (cheap queue issues; the DMAs
        # drain long before stage F's scatters)
        zt = const_pool.tile([P, DIM], BF)
        nc.vector.memset(zt[:], 0.0)
        for i in range(NSPLIT):
            for sc_ in range(NCAP):
                nc.scalar.dma_start(out=ebufs[i][_ts(sc_, P), :], in_=zt[:])

        def ropeT(eng, ps, pool, span, outT):  # ps must be SBUF for gpsimd
            """RoPE in transposed (re,im)-permuted space; write fp8 outT.

            sinT carries [+sin, -sin] per 32-row half-block, so the
            swapped product lands pre-signed and every tensor_tensor has
            partition-aligned inputs (BIR requirement):
              out = ps*cos + swap32(ps)*sin_signed
            """
            sl = _ts(span, 512)
            tc_ = pool.tile([P, 512], F32, tag="rp_c")
            ts_ = pool.tile([P, 512], F32, tag="rp_s")
            eng.tensor_tensor(out=tc_[:], in0=ps, in1=cosT[:, sl], op=OP.mult)
            for h in range(NHL):
                re = slice(h * 64, h * 64 + 32)
                im = slice(h * 64 + 32, h * 64 + 64)
                eng.tensor_tensor(out=ts_[re, :], in0=ps[im, :],
                                  in1=sinT[im, sl], op=OP.mult)
                eng.tensor_tensor(out=ts_[im, :], in0=ps[re, :],
                                  in1=sinT[re, sl], op=OP.mult)
            eng.tensor_tensor(out=outT[:, sl], in0=tc_[:], in1=ts_[:],
                              op=OP.add)

        with tc.tile_pool(name="stA", bufs=3) as pa, \
             tc.tile_pool(name="stB", bufs=3) as pb, \
             tc.tile_pool(name="stC", bufs=3) as pc, \
             tc.tile_pool(name="st_ps", bufs=2, space="PSUM") as ps_pool, \
             tc.tile_pool(name="st_pst", bufs=2, space="PSUM") as pst_pool, \
             tc.tile_pool(name="st_av", bufs=1, space="PSUM") as av_pool:

            def projT(w_sb, span):
                ps = ps_pool.tile([P, 1024], F32, space="PSUM", tag="ps")
                w3 = w_sb[:].rearrange("p (j d) -> p j d", j=ND)
                for j in range(ND):
                    nc.tensor.matmul(
                        out=ps[:, 0:512], lhsT=w3[:, j, :],
                        rhs=xnT3[:, j, _ts(span, 512)],
                        start=(j == 0), stop=(j == ND - 1))
                return ps

            for b in range(B_FULL):
                for s in range(NSB):
                    ssl = _ts(s, 512)
                    # --- A: norm for the 4 token tiles of span s
                    xts, diags = [], []
                    for u in range(4):
                        tg = b * NTB + 4 * s + u
                        xt = pa.tile([P, DIM], BF, tag=f"xt{u}")
                        nc.sync.dma_start(out=xt[:], in_=xbf_in[_ts(tg, P), :])
                        sq = pa.tile([P, DIM], F32, tag="sq")
                        nc.gpsimd.tensor_tensor(out=sq[:], in0=xt[:],
                                                in1=xt[:], op=OP.mult)
                        ssq = pa.tile([P, 1], F32, tag="ssq")
                        nc.vector.reduce_sum(out=ssq[:], in_=sq[:], axis=AX.X)
                        rms = pa.tile([P, 1], F32, tag="rms")
                        nc.scalar.activation(out=rms[:], in_=ssq[:],
                                             func=ACTF.Sqrt,
                                             scale=1.0 / DIM, bias=eps_col[:])
                        rr = pa.tile([P, 1], F32, tag="rr")
                        nc.vector.reciprocal(out=rr[:], in_=rms[:])
                        diag = pa.tile([P, P], BF, tag=f"diag{u}")
                        nc.vector.tensor_scalar_mul(diag[:], id_bf[:], rr[:])
                        xts.append(xt)
                        diags.append(diag)
                    # transposes grouped per j: 4 tiles -> one [128,512] copy
                    for j in range(ND):
                        pst = pst_pool.tile([P, 512], BF, space="PSUM",
                                            tag="pstA")
                        for u in range(4):
                            nc.tensor.transpose(out=pst[:, _ts(u, P)],
                                                in_=xts[u][:, _ts(j, P)],
                                                identity=diags[u][:])
                        nc.vector.tensor_copy(out=xnT3[:, j, ssl], in_=pst[:])
                    # --- B: K/Q (rope'd, transposed) + V for span s
                    ps = projT(wk_sb, s)
                    ropeT(nc.vector, ps[:, 0:512], pb, s, kT)
                    ps = projT(wq_sb, s)
                    qf = pb.tile([P, 512], F32, tag="qf")
                    nc.scalar.activation(out=qf[:], in_=ps[:, 0:512],
                                         func=ACTF.Copy)
                    ropeT(nc.gpsimd, qf[:], pb, s, qT)
                    ps = projT(wv_sb, s)
                    vT = pb.tile([P, 512], BF, tag="vT")
                    nc.scalar.activation(out=vT[:], in_=ps[:, 0:512],
                                         func=ACTF.Copy)
                    pst = pst_pool.tile([P, 512], BF, space="PSUM", tag="pstA")
                    for u in range(4):
                        nc.tensor.transpose(out=pst[:, _ts(u, P)],
                                            in_=vT[:, _ts(u, P)],
                                            identity=id_bf[:])
                    # strided copy: psum (u h d) -> vaug (u [h d |1])
                    va4 = vaug[:, s * 4 * VW:(s + 1) * 4 * VW].rearrange(
                        "p (u h d) -> p u h d", u=4, h=NHL)
                    pst4 = pst[:].rearrange("p (u h d) -> p u h d", u=4, h=NHL)
                    nc.vector.tensor_copy(out=va4[:, :, :, 0:HD], in_=pst4)
                    nc.vector.memset(va4[:, :, :, HD:HD + 1], 1.0)

                # --- C: attention for batch b (2 heads, 2 query halves)
                for h in range(NHL):
                    hsl = slice(h * HD, (h + 1) * HD)
                    for qh in range(2):
                        aug = av_pool.tile([HD + 1, 1024], F32, space="PSUM",
                                           tag="aug")

                        def scores(kt):
                            sps = ps_pool.tile([P, 1024], F32, space="PSUM",
                                               tag="ps")
                            for u in range(2):
                                nc.tensor.matmul(
                                    out=sps[:, _ts(u, 512)],
                                    lhsT=kT[hsl, _ts(kt, P)],
                                    rhs=qT[hsl, qh * 1024 + u * 512:
                                           qh * 1024 + (u + 1) * 512],
                                    start=True, stop=True)
                            ex = pc.tile([P, 1024], BF, tag="expT")
                            nc.scalar.activation(out=ex[:], in_=sps[:],
                                                 func=ACTF.Exp)
                            return ex

                        def av(kt, ex):
                            t0 = kt * VW + h * (HD + 1)
                            for u in range(2):
                                nc.tensor.matmul(
                                    out=aug[:, _ts(u, 512)],
                                    lhsT=vaug[:, t0:t0 + HD + 1],
                                    rhs=ex[:, _ts(u, 512)],
                                    start=(kt == 0), stop=(kt == NTB - 1))

                        # software-pipelined: scores(kt+1) issued before
                        # av(kt) so the PE never stalls on the exp
                        prev = scores(0)
                        for kt in range(1, NTB):
                            cur = scores(kt)
                            av(kt - 1, prev)
                            prev = cur
                        av(NTB - 1, prev)
                        rcp = pc.tile([1, 1024], F32, tag="rcp")
                        nc.vector.reciprocal(out=rcp[:], in_=aug[HD:HD + 1, :])
                        rbc = pc.tile([HD, 1024], F32, tag="rbc")
                        nc.gpsimd.partition_broadcast(rbc[:], rcp[:])
                        nc.vector.tensor_tensor(
                            out=attT_sb[hsl, b * S_FULL + qh * 1024:
                                        b * S_FULL + qh * 1024 + 1024],
                            in0=aug[0:HD, :], in1=rbc[:], op=OP.mult)
        sc_xnT.close()
        nc.leave_named_scope("ABC_attn", scAB[0], False)

        # =============== attention AllToAll ==========================
        scA2A = nc.enter_named_scope("A2A_att", False)
        for c in range(n_cores):
            nc.sync.dma_start(out=att_loc.ap()[_ts(c, P), :],
                              in_=attT_sb[:, _ts(c, LQ)])
        nc.gpsimd.collective_compute(
            "AllToAll", OP.bypass, replica_groups=all_groups,
            ins=[att_loc.ap().opt()], outs=[att_recv.ap().opt()])
        attd = p_att.tile([P, ND * LQ], BF, name="attd", tag="attd")
        attd3 = attd[:].rearrange("p (j t) -> p j t", j=ND)
        for j in range(ND):
            nc.sync.dma_start(out=attd3[:, j, :], in_=att_recv[_ts(j, P), :])
        sc_kv.close()
        nc.leave_named_scope("A2A_att", scA2A[0], False)

        def load_big(pool, src, tag, cols, dt=E4):
            w = pool.tile([P, ND * cols], dt, name=tag, tag=tag)
            nc.sync.dma_start(out=w[:], in_=src[:, :])
            return w[:].rearrange("p (j c) -> p j c", j=ND)

        # =============== stage D: O-proj, gate, hn ===================
        scD = nc.enter_named_scope("D_oproj", False)
        with tc.tile_pool(name="stD", bufs=3) as pd, \
             tc.tile_pool(name="stD_w", bufs=1) as pdw, \
             tc.tile_pool(name="stD_ps", bufs=2, space="PSUM") as pd_ps, \
             tc.tile_pool(name="stD_gps", bufs=2, space="PSUM") as pd_gps, \
             tc.tile_pool(name="stD_pst", bufs=2, space="PSUM") as pd_pst, \
             tc.tile_pool(name="stD_hT", bufs=1) as pd_hT:
            wo3 = load_big(pdw, wo_in, "wo", DIM, dt=BF)
            gate_sb = pdw.tile([P, ND * E], F32, name="g32", tag="g32")
            nc.sync.dma_start(out=gate_sb[:], in_=gate_in[:, :])
            gate3 = gate_sb[:].rearrange("p (j e) -> p j e", j=ND)
            hT32 = [pd_hT.tile([P, LQ], F32, name=f"hT{j}", tag=f"hT{j}")
                    for j in range(ND)]
            hnT3 = hnT[:].rearrange("p (j t) -> p j t", j=ND)
            for t in range(NQ):
                ps = pd_ps.tile([P, DIM], F32, space="PSUM", tag="ops")
                for half in range(2):
                    for j in range(ND):
                        nc.tensor.matmul(
                            out=ps[:, _ts(half, 512)],
                            lhsT=attd3[:, j, _ts(t, P)],
                            rhs=wo3[:, j, _ts(half, 512)],
                            start=(j == 0), stop=(j == ND - 1))
                xres = pd.tile([P, DIM], F32, tag="xres")
                nc.sync.dma_start(out=xres[:], in_=x_in[_ts(t, P), :])
                nc.vector.tensor_tensor(out=h_sb[t][:], in0=ps[:],
                                        in1=xres[:], op=OP.add)
                for j in range(ND):
                    pst = pd_pst.tile([P, P], F32, space="PSUM", tag="pstD")
                    nc.tensor.transpose(out=pst[:], in_=h_sb[t][:, _ts(j, P)],
                                        identity=id_f32[:])
                    nc.vector.tensor_copy(out=hT32[j][:, _ts(t, P)], in_=pst[:])
            # fp32 gate logits first (so the tiny logits AllGather can
            # fire early and routing overlaps the hn AllGather)
            for t in range(NQ):
                gps = pd_gps.tile([P, E], F32, space="PSUM", tag="gps")
                for j in range(ND):
                    nc.tensor.matmul(out=gps[:], lhsT=hT32[j][:, _ts(t, P)],
                                     rhs=gate3[:, j, :],
                                     start=(j == 0), stop=(j == ND - 1))
                sq = pd.tile([P, DIM], F32, tag="sqD")
                ssq = pd.tile([P, 1], F32, tag="ssqD")
                nc.scalar.activation(out=sq[:], in_=h_sb[t][:], func=ACTF.Square,
                                     accum_out=ssq[:])
                rms = pd.tile([P, 1], F32, tag="rmsD")
                nc.scalar.activation(out=rms[:], in_=ssq[:], func=ACTF.Sqrt,
                                     scale=1.0 / DIM, bias=eps_col[:])
                nc.vector.reciprocal(out=rr_sb[t][:], in_=rms[:])
                lg = pd.tile([P, E], F32, tag="lg")
                nc.vector.tensor_scalar_mul(lg[:], gps[:], rr_sb[t][:])
                nc.sync.dma_start(out=lg_loc[_ts(t, P), :], in_=lg[:])
            nc.gpsimd.collective_compute(
                "AllGather", OP.bypass, replica_groups=all_groups,
                ins=[lg_loc.ap().opt()], outs=[lg_full.ap().opt()])
            for t in range(NQ):
                hn = pd.tile([P, DIM], BF, tag="hnD")
                nc.scalar.activation(out=hn[:], in_=h_sb[t][:], func=ACTF.Copy,
                                     scale=rr_sb[t][:])
                nc.sync.dma_start(out=hn_loc[_ts(t, P), :], in_=hn[:])
                for j in range(ND):
                    pst = pd_pst.tile([P, P], BF, space="PSUM", tag="pstD")
                    nc.tensor.transpose(out=pst[:], in_=hn[:, _ts(j, P)],
                                        identity=id_bf[:])
                    nc.vector.tensor_copy(out=hnT3[:, j, _ts(t, P)], in_=pst[:])
        sc_att.close()
        nc.leave_named_scope("D_oproj", scD[0], False)

        # =============== hn AllGather ================================
        scCC = nc.enter_named_scope("CC_gather", False)
        nc.gpsimd.collective_compute(
            "AllGather", OP.bypass, replica_groups=all_groups,
            ins=[hn_loc.ap().opt()], outs=[hn_full.ap().opt()])
        nc.leave_named_scope("CC_gather", scCC[0], False)

        # =============== stage H: shared expert + local output =======
        # (independent of the gathers: its TensorE work fills the window)
        scH = nc.enter_named_scope("H_shared", False)
        with tc.tile_pool(name="stH", bufs=3) as ph, \
             tc.tile_pool(name="stH_w", bufs=1) as phw, \
             tc.tile_pool(name="stH_gT", bufs=1) as ph_gT, \
             tc.tile_pool(name="stH_ps", bufs=2, space="PSUM") as ph_ps:
            s1_3 = load_big(phw, sw1_in, "s1", HID, dt=BF)
            s3_3 = load_big(phw, sw3_in, "s3", HID, dt=BF)
            gsT = ph_gT.tile([P, ND * LQ], BF, name="gsT", tag="gsT")
            gsT3 = gsT[:].rearrange("p (j t) -> p j t", j=ND)
            hnT3 = hnT[:].rearrange("p (j t) -> p j t", j=ND)
            for j in range(ND):
                h1 = ph_ps.tile([P, LQ], F32, space="PSUM", tag="sh1")
                h3 = ph_ps.tile([P, LQ], F32, space="PSUM", tag="sh3")
                for d in range(ND):
                    nc.tensor.matmul(out=h1[:],
                                     lhsT=s1_3[:, d, _ts(j, P)],
                                     rhs=hnT3[:, d, :],
                                     start=(d == 0), stop=(d == ND - 1))
                for d in range(ND):
                    nc.tensor.matmul(out=h3[:],
                                     lhsT=s3_3[:, d, _ts(j, P)],
                                     rhs=hnT3[:, d, :],
                                     start=(d == 0), stop=(d == ND - 1))
                sig = ph.tile([P, LQ], F32, tag="sigH")
                nc.scalar.activation(out=sig[:], in_=h1[:], func=ACTF.Sigmoid)
                nc.vector.tensor_tensor(out=sig[:], in0=sig[:], in1=h1[:],
                                        op=OP.mult)
                nc.vector.tensor_tensor(out=gsT3[:, j, :], in0=sig[:],
                                        in1=h3[:], op=OP.mult)
            s2_3 = load_big(phw, sw2_in, "s2", DIM, dt=BF)
            for t in range(NQ):
                ps = ph_ps.tile([P, DIM], F32, space="PSUM", tag="shps")
                for half in range(2):
                    for j in range(ND):
                        nc.tensor.matmul(
                            out=ps[:, _ts(half, 512)],
                            lhsT=gsT3[:, j, _ts(t, P)],
                            rhs=s2_3[:, j, _ts(half, 512)],
                            start=(j == 0), stop=(j == ND - 1))
                ot = ph.tile([P, DIM], F32, tag="ot")
                nc.vector.tensor_tensor(out=ot[:], in0=ps[:], in1=h_sb[t][:],
                                        op=OP.add)
                nc.sync.dma_start(out=out_local[_ts(t, P), :], in_=ot[:])
        nc.leave_named_scope("H_shared", scH[0], False)

        # =============== stage F: routing + dispatch =================
        # Selection on raw fp32 logits (host replicates it from lg_out).
        scF = nc.enter_named_scope("F_route", False)
        with tc.tile_pool(name="stF", bufs=8) as pf, \
             tc.tile_pool(name="stF_keep", bufs=1) as pfk, \
             tc.tile_pool(name="stF_ps", bufs=2, space="PSUM") as pf_ps, \
             tc.tile_pool(name="stF_tot", bufs=1, space="PSUM") as pf_tot:
            lg_all = pfk.tile([P, NA * E], F32)
            nc.sync.dma_start(
                out=lg_all[:].rearrange("p (t e) -> p t e", t=NA),
                in_=lg_full.ap().rearrange("(t p) e -> p t e", p=P))
            nc.sync.dma_start(
                out=lg_out.ap().rearrange("(t p) e -> p t e", p=P),
                in_=lg_all[:].rearrange("p (t e) -> p t e", t=NA))
            v3 = lg_all[:].rearrange("p (t e) -> p t e", t=NA)
            m1 = pfk.tile([P, NA], F32)
            nc.vector.reduce_max(out=m1[:], in_=v3, axis=AX.X)
            ge1 = pfk.tile([P, NA * E], F32)
            g13 = ge1[:].rearrange("p (t e) -> p t e", t=NA)
            nc.vector.tensor_tensor(out=g13, in0=v3,
                                    in1=m1[:, :, None].to_broadcast([P, NA, E]),
                                    op=OP.is_ge)
            msk = pfk.tile([P, NA * E], F32)
            nc.vector.tensor_scalar_mul(msk[:], ge1[:], -1.0e30)
            nc.vector.tensor_tensor(out=msk[:], in0=msk[:], in1=lg_all[:],
                                    op=OP.add)
            m2 = pfk.tile([P, NA], F32)
            nc.vector.reduce_max(out=m2[:],
                                 in_=msk[:].rearrange("p (t e) -> p t e", t=NA),
                                 axis=AX.X)
            ge = pfk.tile([P, NA * E], F32)
            ge3 = ge[:].rearrange("p (t e) -> p t e", t=NA)
            nc.vector.tensor_tensor(out=ge3, in0=v3,
                                    in1=m2[:, :, None].to_broadcast([P, NA, E]),
                                    op=OP.is_ge)
            msel = pfk.tile([P, NA * E], F32)
            nc.vector.tensor_tensor(out=msel[:], in0=ge[:], in1=oh_bc[:],
                                    op=OP.mult)
            ind = pfk.tile([P, NA], F32)
            nc.vector.reduce_sum(out=ind[:],
                                 in_=msel[:].rearrange("p (t e) -> p t e", t=NA),
                                 axis=AX.X)
            # per-tile totals + within-tile inclusive cumsum: one matmul each
            tots = pf_tot.tile([1, NA], F32, space="PSUM")
            nc.tensor.matmul(out=tots[:], lhsT=ones_col[:], rhs=ind[:],
                             start=True, stop=True)
            cnts = pf_tot.tile([P, NA], F32, space="PSUM")
            nc.tensor.matmul(out=cnts[:], lhsT=ltri[:], rhs=ind[:],
                             start=True, stop=True)
            # batched exclusive cumsum of tile totals -> per-tile bases
            tots_sb = pf.tile([1, NA], F32, tag="tots_sb")
            nc.vector.tensor_copy(out=tots_sb[:], in_=tots[:])
            totsT_ps = pf_ps.tile([NA, 1], F32, space="PSUM", tag="totsT", bufs=1)
            nc.tensor.transpose(out=totsT_ps[:], in_=tots_sb[:],
                                identity=id_f32[:1, :1])
            totsT = pf.tile([NA, 1], F32, tag="totsT_sb")
            nc.vector.tensor_copy(out=totsT[:], in_=totsT_ps[:])
            basesT_ps = pf_ps.tile([NA, 1], F32, space="PSUM", tag="basesT", bufs=1)
            nc.tensor.matmul(out=basesT_ps[:], lhsT=ltri_s[:NA, :NA],
                             rhs=totsT[:], start=True, stop=True)
            basesT = pf.tile([NA, 1], F32, tag="basesT_sb")
            nc.vector.tensor_copy(out=basesT[:], in_=basesT_ps[:])
            bases_ps = pf_ps.tile([1, NA], F32, space="PSUM", tag="bases", bufs=1)
            nc.tensor.transpose(out=bases_ps[:], in_=basesT[:],
                                identity=id_f32[:NA, :NA])
            bases_sb = pf.tile([1, NA], F32, tag="bases_sb")
            nc.vector.tensor_copy(out=bases_sb[:], in_=bases_ps[:])
            bb_all = pfk.tile([P, NA], F32)
            nc.gpsimd.partition_broadcast(bb_all[:], bases_sb[:])
            # destinations (batched)
            d_all = pfk.tile([P, NA], F32)
            nc.vector.scalar_tensor_tensor(
                out=d_all[:], in0=cnts[:], scalar=-(1.0 + BIG),
                in1=bb_all[:], op0=OP.add, op1=OP.add)
            nc.vector.tensor_tensor(out=d_all[:], in0=d_all[:], in1=ind[:],
                                    op=OP.mult)
            nc.vector.tensor_scalar_add(d_all[:], d_all[:], BIG)
            dest_all = pfk.tile([P, NA], I32)
            nc.vector.tensor_copy(out=dest_all[:], in_=d_all[:])
            # scatters (independent per tile)
            for t in range(NA):
                hnt = pf.tile([P, DIM], BF, tag="hnF")
                nc.sync.dma_start(out=hnt[:], in_=hn_full[_ts(t, P), :])
                nc.gpsimd.indirect_dma_start(
                    out=ebufs[t % NSPLIT][:, :],
                    out_offset=bass.IndirectOffsetOnAxis(
                        ap=dest_all[:, t:t + 1], axis=0),
                    in_=hnt[:], in_offset=None,
                    bounds_check=C - 1, oob_is_err=False)
        nc.leave_named_scope("F_route", scF[0], False)

        # =============== stage G: expert FFN =========================
        scG = nc.enter_named_scope("G_expert", False)
        with tc.tile_pool(name="stG", bufs=3) as pg, \
             tc.tile_pool(name="stG_w", bufs=1) as pgw, \
             tc.tile_pool(name="stG_gT", bufs=1) as pg_gT:
            ebT = pg_gT.tile([P, ND * C], E4, name="ebT", tag="ebT")
            ebT3 = ebT[:].rearrange("p (j t) -> p j t", j=ND)
            with tc.tile_pool(name="stG_ps", bufs=4, space="PSUM") as pg_ps:
                for s in range(NCAP):
                    parts = []
                    for i in range(NSPLIT):
                        pt = pg.tile([P, DIM], BF, tag=f"ebp{i}", bufs=2)
                        nc.sync.dma_start(out=pt[:], in_=ebufs[i][_ts(s, P), :])
                        parts.append(pt)
                    nc.vector.tensor_tensor(out=parts[0][:], in0=parts[0][:],
                                            in1=parts[1][:], op=OP.add)
                    nc.vector.tensor_tensor(out=parts[2][:], in0=parts[2][:],
                                            in1=parts[3][:], op=OP.add)
                    eb = pg.tile([P, DIM], BF, tag="eb")
                    nc.vector.tensor_tensor(out=eb[:], in0=parts[0][:],
                                            in1=parts[2][:], op=OP.add)
                    for j in range(ND):
                        pst = pg_ps.tile([P, P], BF, space="PSUM", tag="pstG")
                        nc.tensor.transpose(out=pst[:], in_=eb[:, _ts(j, P)],
                                            identity=id_bf[:])
                        nc.vector.tensor_copy(out=ebT3[:, j, _ts(s, P)],
                                              in_=pst[:])
            e1_3 = load_big(pgw, ew1_in, "e1", HID)
            e3_3 = load_big(pgw, ew3_in, "e3", HID)
            gT = pg_gT.tile([P, ND * C], E4, name="gT", tag="gT")
            gT3 = gT[:].rearrange("p (j t) -> p j t", j=ND)
            nsub = (C + 511) // 512
            with tc.tile_pool(name="stG_ps2", bufs=2, space="PSUM") as pg_ps2:
                for j in range(ND):
                    for s in range(nsub):
                        w = min(512, C - s * 512)
                        sl = slice(s * 512, s * 512 + w)
                        h1 = pg_ps2.tile([P, 512], F32, space="PSUM", tag="h1")
                        h3 = pg_ps2.tile([P, 512], F32, space="PSUM", tag="h3")
                        for d in range(ND // 2):
                            nc.tensor.matmul(
                                out=h1[:, :w],
                                lhsT=e1_3[:, 2 * d:2 * d + 2, _ts(j, P)],
                                rhs=ebT3[:, 2 * d:2 * d + 2, sl],
                                start=(d == 0), stop=(d == ND // 2 - 1),
                                perf_mode=DR)
                        for d in range(ND // 2):
                            nc.tensor.matmul(
                                out=h3[:, :w],
                                lhsT=e3_3[:, 2 * d:2 * d + 2, _ts(j, P)],
                                rhs=ebT3[:, 2 * d:2 * d + 2, sl],
                                start=(d == 0), stop=(d == ND // 2 - 1),
                                perf_mode=DR)
                        sig = pg.tile([P, 512], F32, tag="sig")
                        nc.scalar.activation(out=sig[:, :w], in_=h1[:, :w],
                                             func=ACTF.Sigmoid, scale=1.0 / SW)
                        nc.vector.tensor_tensor(out=sig[:, :w], in0=sig[:, :w],
                                                in1=h1[:, :w], op=OP.mult)
                        nc.vector.scalar_tensor_tensor(
                            out=gT3[:, j, sl], in0=sig[:, :w],
                            scalar=1.0 / (SW * SW), in1=h3[:, :w],
                            op0=OP.mult, op1=OP.mult)
                e2_3 = load_big(pgw, ew2_in, "e2", DIM)
                for s in range(NCAP):
                    ps = pg_ps2.tile([P, DIM], F32, space="PSUM", tag="eops")
                    for half in range(2):
                        for j in range(ND // 2):
                            nc.tensor.matmul(
                                out=ps[:, _ts(half, 512)],
                                lhsT=gT3[:, 2 * j:2 * j + 2, _ts(s, P)],
                                rhs=e2_3[:, 2 * j:2 * j + 2, _ts(half, 512)],
                                start=(j == 0), stop=(j == ND // 2 - 1),
                                perf_mode=DR)
                    eo = pg.tile([P, DIM], F32, tag="eo")
                    nc.scalar.activation(out=eo[:], in_=ps[:], func=ACTF.Copy,
                                         scale=1.0 / SW)
                    nc.sync.dma_start(out=eo_out[_ts(s, P), :], in_=eo[:])
        nc.leave_named_scope("G_expert", scG[0], False)

    nc.compile()
    return nc


# ----------------------------------------------------------------------
# host side
# ----------------------------------------------------------------------

def _tile128(w):
    """[R, C] -> [128, (R/128)*C] with row-tile-major layout."""
    R, Cc = w.shape
    return np.ascontiguousarray(
        w.reshape(R // P, P, Cc).transpose(1, 0, 2).reshape(P, (R // P) * Cc))


def prep_inputs(x, freqs, att_norm_w, wq, wk, wv, wo, ffn_norm_w, gate_w,
                ew1, ew2, ew3, sw1, sw2, sw3, LQ=LQ_FULL, n_cores=8):
    """Build the 8 per-core input maps (host-side weight folding + slicing)."""
    def to8(a):
        return np.ascontiguousarray(np.asarray(a, np.float32).astype(FP8))

    def tobf(a):
        return np.ascontiguousarray(np.asarray(a, np.float32).astype(BF16))

    B, S, _ = x.shape
    N = B * S
    anw = np.asarray(att_norm_w, np.float32)
    fnw = np.asarray(ffn_norm_w, np.float32)
    wq_n = (anw[:, None] * np.asarray(wq, np.float32)) / np.sqrt(HD)
    wk_n = anw[:, None] * np.asarray(wk, np.float32)
    wv_n = anw[:, None] * np.asarray(wv, np.float32)
    wo_e = tobf(_tile128(np.asarray(wo, np.float32)))
    gate32 = np.ascontiguousarray(
        _tile128((np.asarray(gate_w, np.float32) * fnw[None, :]).T))
    ew1_e = np.asarray(ew1, np.float32) * fnw[None, :, None] * SW
    ew3_e = np.asarray(ew3, np.float32) * fnw[None, :, None] * SW
    ew2_e = np.asarray(ew2, np.float32) * SW
    sw1_e = tobf(_tile128(np.asarray(sw1, np.float32) * fnw[:, None]))
    sw3_e = tobf(_tile128(np.asarray(sw3, np.float32) * fnw[:, None]))
    sw2_e = tobf(_tile128(np.asarray(sw2, np.float32)))

    x_flat = np.asarray(x, np.float32).reshape(N, DIM)
    x_bf = np.ascontiguousarray(x_flat.astype(BF16))
    # rope tables in transposed space: row r -> pair (r % 32)
    cos32 = np.asarray(freqs[:S, :, 0], np.float32).T        # (32, S)
    sin32 = np.asarray(freqs[:S, :, 1], np.float32).T
    cosT = np.ascontiguousarray(np.tile(cos32, (4, 1)))      # (128, S)
    # signed sin: +sin on re rows (0-31 of each head block), -sin on im rows
    sinT = np.ascontiguousarray(np.tile(np.vstack([sin32, -sin32]), (2, 1)))
    # within-head (re, im) column permutation for transposed-space rope
    pidx = np.concatenate([np.arange(0, HD, 2), np.arange(1, HD, 2)])

    in_maps = []
    for core in range(n_cores):
        heads = [2 * core, 2 * core + 1]
        wq_c = np.hstack([wq_n[:, h * HD + pidx] for h in heads])
        wk_c = np.hstack([wk_n[:, h * HD + pidx] for h in heads])
        wv_c = np.hstack([wv_n[:, _ts(h, HD)] for h in heads])
        na = N // 128
        oh = np.zeros((1, E), np.float32)
        oh[0, core % E] = 1.0
        oh = np.tile(oh, (1, na))
        in_maps.append(dict(
            x_bf=x_bf,
            x_chunk=np.ascontiguousarray(x_flat[core * LQ:(core + 1) * LQ]),
            cosT=cosT, sinT=sinT,
            wq_t8=tobf(_tile128(wq_c)), wk_t8=tobf(_tile128(wk_c)),
            wv_t8=tobf(_tile128(wv_c)),
            wo_t8=wo_e, gate_t32=gate32,
            sw1_t8=sw1_e, sw2_t8=sw2_e, sw3_t8=sw3_e,
            ew1_t8=to8(_tile128(ew1_e[core % E])),
            ew2_t8=to8(_tile128(ew2_e[core % E])),
            ew3_t8=to8(_tile128(ew3_e[core % E])),
            onehot=oh,
        ))
    return in_maps


def assemble(results, B, S, LQ=LQ_FULL, n_cores=8):
    N = B * S
    out = np.zeros((N, DIM), np.float32)
    y = np.zeros((N, DIM), np.float32)
    # replicate the device's top-2 selection exactly from the fp32 logits
    lg = np.asarray(results[0]["lg_out"], np.float32)          # (N, E)
    m2 = np.partition(lg, -2, axis=1)[:, -2]
    sel_mask = lg >= m2[:, None]
    ex = np.exp(lg - lg.max(axis=1, keepdims=True), dtype=np.float32)
    probs = ex / ex.sum(axis=1, keepdims=True, dtype=np.float32)
    for core, res in enumerate(results):
        tok0 = core * LQ
        out[tok0:tok0 + LQ] = res["out_local"]
        e = core % E
        sel = np.nonzero(sel_mask[:, e])[0]
        cnt = len(sel)
        eo = res["eo_out"]
        assert cnt <= eo.shape[0], (core, cnt)
        y[sel] += probs[sel, e:e + 1] * eo[:cnt]
    return (out + y).reshape(B, S, DIM)


_NC_CACHE = {}


def kernel(**inputs):
    key = "full"
    if key not in _NC_CACHE:
        _NC_CACHE[key] = build_nc()
    nc = _NC_CACHE[key]
    from concourse.bass_utils import run_bass_kernel_spmd
    in_maps = prep_inputs(**inputs)
    res = run_bass_kernel_spmd(nc, in_maps, core_ids=list(range(8)))
    x = np.asarray(inputs["x"])
    return assemble(res.results, x.shape[0], x.shape[1]).astype(np.float32)


if __name__ == "__main__":
    nc = build_nc()
    print("built + compiled OK")
